# revision 1
# baseline (speedup 1.0000x reference)
"""DeepSeek-style LM on 8 TRN2 cores — tensor-parallel Bass/Tile kernel.

Sharding: 2 Q heads + 1 KV head per core, HFF/8 per core, V/8 per core,
token-block/8 per core for residual+norm. Cross-core comm via
remote_dma_broadcast (SBUF->SBUF). Activations feature-major [C_part, T].

v2: pipelined communication — sends fire as soon as chunks are computed
(y2 per tau-pair, W3 reduce-scatter per slot in slot order), arrival
waits deferred to just before the first consumer. No credit handshake
(global round order transitively protects buffer reuse).
"""

import numpy as np
import ml_dtypes
from contextlib import ExitStack
from einops import rearrange

import concourse.bass as bass
import concourse.tile as tile
from concourse import bacc, mybir
from concourse.bass import ds
from concourse.masks import make_identity

F32 = mybir.dt.float32
BF16 = mybir.dt.bfloat16
I32 = mybir.dt.int32

NCORES = 8
T, C, DH, L, V, HFF = 2048, 1024, 64, 4, 32000, 4096
TLOC = T // NCORES          # 256
VLOC = V // NCORES          # 4000
EPS = 1e-5
AGB = 8 * TLOC              # 2048 elems: one sender's block in xh/rs_recv

# jax device index -> physical NeuronCore (tpb) index on this chip.
# remote_dma relative addressing XORs PHYSICAL tpb ids, so per-slot logical
# destinations are P^-1(P(c) ^ slot).
PHYS = [4, 5, 6, 7, 2, 3, 0, 1]
PHYS_INV = [PHYS.index(i) for i in range(8)]


def build_nc(taps=()):
    nc = bacc.Bacc("TRN2", target_bir_lowering=False, debug=False,
                   num_devices=NCORES)

    # ---------------- DRAM I/O ----------------
    dt = nc.dram_tensor
    wq_d = dt("wq", [L, 128, 8, 128], BF16, kind="ExternalInput").ap()
    wkv_d = dt("wkv", [L, 128, 8, 128], BF16, kind="ExternalInput").ap()
    wo_d = dt("wo", [L, 128, 8, 8, 128], BF16, kind="ExternalInput").ap()
    w12_d = dt("w12", [L, 128, 8, 8, 128], BF16, kind="ExternalInput").ap()
    w3_d = dt("w3", [L, 128, 4, 8, 128], BF16, kind="ExternalInput").ap()
    embt_d = dt("embt", [128, 8, VLOC], BF16, kind="ExternalInput").ap()
    x0_d = dt("x0", [2, 128, C], F32, kind="ExternalInput").ap()
    rope_d = dt("rope", [128, T], BF16, kind="ExternalInput").ap()
    dmask_d = dt("dmask", [128, 896], BF16, kind="ExternalInput").ap()
    info_d = dt("coreinfo", [1, 32], I32, kind="ExternalInput").ap()
    logits_d = dt("logits", [T, VLOC], F32, kind="ExternalOutput").ap()
    tap_d = {}
    for t_ in taps:
        shp = {"xh": [128, NCORES * AGB], "qfm": [128, T], "k2": [128, T],
               "vaug": [128, 16 * 65], "y2": [128, T],
               "xres": [128, 8 * TLOC], "hfm": [128, 4 * T],
               "rsrecv": [128, NCORES * AGB],
               "xres2": [128, 8 * TLOC]}[t_]
        dtp = F32 if t_ in ('xres', 'xres2') else BF16
        tap_d[t_] = dt("tap_" + t_, shp, dtp, kind="ExternalOutput").ap()

    # ---------------- static SBUF (fixed addrs for remote writes) -------
    xh = nc.alloc_sbuf_tensor("xh", [128, NCORES * AGB], BF16).ap()
    rs_recv = nc.alloc_sbuf_tensor("rs_recv", [128, NCORES * AGB], BF16).ap()
    info_sb = nc.alloc_sbuf_tensor("info_sb", [1, 32], I32).ap()

    xh4 = xh.rearrange("p (c k t) -> p c k t", c=NCORES, k=8)
    rsr = rs_recv.rearrange("p (c t) -> p c t", c=NCORES)

    # ---------------- semaphores ----------------
    sem = nc.alloc_semaphore
    lsem = sem("lsem")
    psem = sem("psem")
    dmas = sem("dmas")
    rsem = {k: sem(f"rsem_{k}") for k in ("xh", "rsb")}
    RD_ALL = [(0, k) for k in range(NCORES)]

    # cumulative counters (python-side bookkeeping of semaphore targets)
    state = dict(preps=0, lsem=0, rs={"xh": 0, "rsb": 0})

    def comm_send(kind, sends, name=""):
        """Issue remote broadcasts; no arrival wait. sends: [(src, dst, slot)]."""
        gp = nc.gpsimd
        with tc.tile_critical(name=name):
            for src, dst, slot in sends:
                rd = RD_ALL if slot is None else \
                    [(0, k) if k == slot else None for k in range(NCORES)]
                gp.remote_dma_broadcast(out_ap=dst, in_ap=src,
                                        remote_sem=rsem[kind], local_sem=lsem,
                                        rdests=rd).then_inc(psem, 1)
                state["preps"] += 1
                state["lsem"] += 16
                state["rs"][kind] += 2 if slot is not None else 16
            gp.wait_ge(psem, state["preps"])
            gp.trigger_dma(count=len(sends))

    def comm_wait(kind, target, lsem_thr=None, name=""):
        """Wait for arrivals (and optionally local send completion).
        Returns the crit's post-crit instruction for add_dep_helper edges
        (the scheduler treats crit bodies as opaque, so edges must target
        the post-crit barrier, not the inner wait)."""
        gp = nc.gpsimd
        with tc.tile_critical(name=name):
            gp.wait_ge(rsem[kind], target)
            if lsem_thr is not None:
                gp.wait_ge(lsem, lsem_thr)
        return tc.prev_crit_insts[mybir.EngineType.Pool]

    def dep(consumer, waiter, why="comm arrival"):
        tile.add_dep_helper(consumer.ins, waiter, True, why)

    with tile.TileContext(nc) as tc, ExitStack() as ctx:
        # ---------- pools ----------
        sing = ctx.enter_context(tc.tile_pool(name="sing", bufs=1))
        spool = ctx.enter_context(tc.tile_pool(name="spool", bufs=1))
        spool1 = ctx.enter_context(tc.tile_pool(name="spool1", bufs=1))
        layer_ctx = ExitStack()
        act = layer_ctx.enter_context(tc.tile_pool(name="act", bufs=1))
        wpool = layer_ctx.enter_context(tc.tile_pool(name="wpool", bufs=2))
        wopool = layer_ctx.enter_context(tc.tile_pool(name="wopool", bufs=2))
        w8pool = layer_ctx.enter_context(tc.tile_pool(name="w8pool", bufs=1))
        w4pool = layer_ctx.enter_context(tc.tile_pool(name="w4pool", bufs=1))
        ppool = layer_ctx.enter_context(tc.tile_pool(name="ppool", bufs=2))
        prtpool = layer_ctx.enter_context(tc.tile_pool(name="prtpool", bufs=2))

        # ---------- constants ----------
        ident = sing.tile([128, 128], F32)
        make_identity(nc, ident)
        ident_bf = sing.tile([64, 64], BF16)
        nc.vector.tensor_copy(ident_bf[:], ident[0:64, 0:64])
        ones_sb = sing.tile([128, 128], BF16)
        nc.vector.memset(ones_sb, 1.0)
        rope_sb = sing.tile([128, T], BF16)
        nc.sync.dma_start(rope_sb[:], rope_d)
        eps_sb = sing.tile([128, 1], F32)
        nc.vector.memset(eps_sb, EPS)
        dmask_sb = sing.tile([128, 896], BF16)
        nc.sync.dma_start(dmask_sb[:], dmask_d)

        # persistent activations
        x_resid = sing.tile([128, 8, TLOC], F32)
        xh_send = sing.tile([128, 8, TLOC], BF16)
        xh_send_f = xh_send[:].rearrange("p k t -> p (k t)")
        x_resid_f = x_resid[:].rearrange("p k t -> p (k t)")

        # ---------- registers from coreinfo ----------
        regs = {}
        with tc.tile_critical():
            nc.gpsimd.dma_start(info_sb, info_d).then_inc(dmas, 16)
            nc.gpsimd.wait_ge(dmas, 16)

            def ld(eng, idx, mx):
                r = eng.alloc_register(f"r{idx}")
                eng.reg_load(r, info_sb[0:1, idx:idx + 1])
                return eng.snap(r, donate=True, min_val=0, max_val=mx)
            regs["xh_slot"] = ld(nc.gpsimd, 0, (NCORES - 1) * AGB)
            regs["y_slot"] = ld(nc.gpsimd, 1, (NCORES - 1) * TLOC)
            regs["xh_slot2"] = ld(nc.gpsimd, 2, (NCORES - 1) * AGB + 1024)
            regs["tok"] = [ld(nc.gpsimd, 8 + d, (NCORES - 1) * TLOC)
                           for d in range(8)]
            # PE-engine copies for reg-offset moving APs (regs are per-engine)
            nc.tensor.wait_ge(dmas, 16)
            regs["y_slot_pe"] = ld(nc.tensor, 1, (NCORES - 1) * TLOC)
            regs["tok_pe"] = [ld(nc.tensor, 8 + d, (NCORES - 1) * TLOC)
                              for d in range(8)]

        # ---------- helpers ----------
        def norm_to_xh_send():
            """xh_send = rmsnorm(x_resid); first writer is a vector mul that
            is ordered behind the preceding comm_wait via vector-queue FIFO."""
            with tc.tile_pool(name="psn", bufs=1, space="PSUM") as psn:
                ns = psn.tile([128, TLOC], F32)
                for k in range(8):
                    sq = spool1.tile([128, TLOC], BF16, tag="sq")
                    nc.vector.tensor_mul(sq[:], x_resid[:, k, :], x_resid[:, k, :])
                    nc.tensor.matmul(ns[:], ones_sb[:], sq[:],
                                     start=(k == 0), stop=(k == 7))
                rms = spool1.tile([128, TLOC], F32, tag="rms")
                nc.scalar.activation(rms[:], ns[:],
                                     mybir.ActivationFunctionType.Sqrt,
                                     bias=eps_sb[:], scale=1.0 / C)
                rin = spool1.tile([128, TLOC], F32, tag="rin")
                nc.vector.reciprocal(rin[:], rms[:])
                for k in range(8):
                    nc.vector.tensor_mul(xh_send[:, k, :], x_resid[:, k, :], rin[:])
                    if k == 3:
                        comm_send("xh", [(xh_send_f[:, 0:1024],
                                          xh[:, ds(regs["xh_slot"], 1024)],
                                          None)], name="agA")

        def ag_send(name):
            comm_send("xh", [(xh_send_f[:, 1024:2048],
                              xh[:, ds(regs["xh_slot2"], 1024)], None)],
                      name=name)

        # ---------- x0 init ----------
        with tc.tile_pool(name="x0p", bufs=2) as x0p:
            with tc.tile_pool(name="ps0", bufs=2, space="PSUM") as ps0:
                for i in range(2):
                    x0_sb = x0p.tile([128, C], F32, tag="x0", name="x0_sb")
                    nc.sync.dma_start(x0_sb[:], x0_d[i])
                    for k in range(8):
                        tp = ps0.tile([128, 128], F32)
                        nc.tensor.transpose(tp[:], x0_sb[:, 128 * k:128 * (k + 1)],
                                            ident[:])
                        nc.vector.tensor_copy(x_resid[:, k, 128 * i:128 * (i + 1)],
                                              tp[:])
        norm_to_xh_send()
        ag_send("ag0")   # round 0: xh <- layer-0 attn input
        state.setdefault("ag_thr", []).append(state["rs"]["xh"])

        def rope_apply(out_fm, ps, base, tau):
            """rotate-half on psum rows [base:base+64] -> out_fm bf16."""
            sl = slice(512 * tau, 512 * (tau + 1))
            cos = rope_sb[base:base + 32, sl]
            sin = rope_sb[base + 32:base + 64, sl]
            x1 = ps[base:base + 32, :]
            x2 = ps[base + 32:base + 64, :]
            t1 = spool.tile([32, 512], F32, tag="rt1")
            t2 = spool.tile([32, 512], F32, tag="rt2")
            nc.vector.tensor_mul(t1[:], x1, cos)
            nc.vector.tensor_mul(t2[:], x2, sin)
            nc.vector.tensor_sub(out_fm[base:base + 32, sl], t1[:], t2[:])
            nc.vector.tensor_mul(t1[:], x1, sin)
            nc.vector.tensor_mul(t2[:], x2, cos)
            nc.vector.tensor_add(out_fm[base + 32:base + 64, sl], t1[:], t2[:])

        # wq/wkv for layer 0 prefetched before the first wait
        wq_t = wpool.tile([128, 8, 128], BF16, tag="wq")
        nc.sync.dma_start(wq_t[:], wq_d[0])
        wkv_t = wpool.tile([128, 8, 128], BF16, tag="wkv")
        nc.sync.dma_start(wkv_t[:], wkv_d[0])

        # ================= layers =================
        for l in range(L):
            # ---- wait for this layer's attn all-gather ----
            wa = comm_wait("xh", state["ag_thr"][-1],
                           lsem_thr=state.get("lsem_prev"), name=f"wA{l}")
            q_fm = act.tile([128, T], BF16, tag="q_fm", name=f"q_fm{l}")
            k2_fm = act.tile([128, T], BF16, tag="k2_fm", name=f"k2_fm{l}")
            v_aug = act.tile([128, 16, 65], BF16, tag="v_aug", name=f"v_aug{l}")
            y2_send = act.tile([128, T], BF16, tag="y2_send", name=f"y2_send{l}")

            # prefetch Wo during QKV/attention (sync queue, before any crit
            # that could trap it behind the y wait; bufs=2 paces the stream)
            wo_t = []
            for cp in range(8):
                w_ = wopool.tile([128, 8, 128], BF16, tag="wo",
                                 name=f"wo_t{l}_{cp}")
                nc.sync.dma_start(w_[:], wo_d[l, :, cp])
                wo_t.append(w_)

            # ---- QKV matmuls back-to-back; v transposes deferred so the PE
            # stream is not interleaved with vector-dependent work ----
            first_qkv = None
            v_stf = act.tile([64, T], BF16, tag="v_stf", name=f"v_stf{l}")
            with tc.tile_pool(name=f"psq{l}", bufs=2, space="PSUM") as P, \
                 tc.tile_pool(name=f"psk{l}", bufs=4, space="PSUM") as PK, \
                 tc.tile_pool(name=f"psv{l}", bufs=2, space="PSUM") as PV:
                for tau in range(4):
                    rhs = xh4[:, 2 * tau:2 * tau + 2, :, :]
                    q_ps = P.tile([128, 512], F32, tag="q")
                    for k in range(8):
                        mm = nc.tensor.matmul(
                            q_ps[:], wq_t[:, k, :], rhs[:, :, k, :],
                            start=(k == 0), stop=(k == 7))
                        if first_qkv is None:
                            first_qkv = mm
                            dep(mm, wa, "attn AG arrival")
                    rope_apply(q_fm, q_ps, 0, tau)
                    rope_apply(q_fm, q_ps, 64, tau)
                    kv_ps = PK.tile([128, 512], F32, tag="k")
                    for k in range(8):
                        nc.tensor.matmul(
                            kv_ps[:], wkv_t[:, k, :], rhs[:, :, k, :],
                            start=(k == 0), stop=(k == 7))
                    rope_apply(k2_fm, kv_ps, 0, tau)
                    # duplicate k rows for head 1 (scalar engine is idle here)
                    nc.scalar.copy(k2_fm[64:128, 512 * tau:512 * (tau + 1)],
                                   k2_fm[0:64, 512 * tau:512 * (tau + 1)])
                    nc.vector.tensor_copy(v_stf[:, 512 * tau:512 * (tau + 1)],
                                          kv_ps[64:128, :])
                for i in range(16):
                    vt = PV.tile([128, 64], BF16, tag="vtr")
                    nc.tensor.transpose(vt[:], v_stf[:, 128 * i:128 * (i + 1)],
                                        ident_bf[0:64, 0:64])
                    nc.vector.tensor_copy(v_aug[:, i, 0:64], vt[:])
                nc.vector.memset(v_aug[:, :, 64:65], 1.0)
            if l == 0 and "qfm" in tap_d:
                nc.sync.dma_start(tap_d["qfm"], q_fm[:])
            if l == 0 and "k2" in tap_d:
                nc.sync.dma_start(tap_d["k2"], k2_fm[:])
            if l == 0 and "vaug" in tap_d:
                nc.sync.dma_start(tap_d["vaug"],
                                  v_aug[:].rearrange("p a b -> p (a b)"))

            # prefetch W1/W2 during attention
            w12_t = [w8pool.tile([128, 8, 128], BF16, tag=f"w12_{k}",
                                 name=f"w12_t{l}_{k}") for k in range(8)]
            for k in range(8):
                nc.sync.dma_start(w12_t[k][:], w12_d[l, :, k])

            # ---- scores + softmax + AV; y2 broadcast per tau-pair ----
            with tc.tile_pool(name=f"psa{l}", bufs=2, space="PSUM") as B:
                for tau in range(4):
                    y_ps = [B.tile([65, 512], F32, tag=f"y{h}",
                                   name=f"y_ps{h}") for h in (0, 1)]
                    na = 4 * tau + 4
                    for a in range(na):
                        pts = []
                        for h in (0, 1):
                            s_ps = B.tile([128, 512], F32, tag=f"s{h}")
                            nc.tensor.matmul(
                                s_ps[:],
                                k2_fm[64 * h:64 * h + 64, 128 * a:128 * (a + 1)],
                                q_fm[64 * h:64 * h + 64, 512 * tau:512 * (tau + 1)],
                                start=True, stop=True)
                            p_t = ppool.tile([128, 512], BF16, tag=f"pT{h}")
                            nc.scalar.activation(p_t[:], s_ps[:],
                                                 mybir.ActivationFunctionType.Exp)
                            if a >= 4 * tau:
                                r_ = a - 4 * tau
                                nc.vector.tensor_mul(
                                    p_t[:], p_t[:],
                                    dmask_sb[:, 384 - 128 * r_:896 - 128 * r_])
                            pts.append(p_t)
                        for h in (0, 1):
                            nc.tensor.matmul(y_ps[h][:], v_aug[:, a, :], pts[h][:],
                                             start=(a == 0), stop=(a == na - 1))
                    # softmax denominators: batch both heads in one reciprocal
                    # (partition_broadcast needs base-0 in/out; gpsimd copies
                    # shift rows to base 0 first)
                    den = spool1.tile([33, 512], F32, tag="den")
                    for h in (0, 1):
                        nc.vector.tensor_copy(den[32 * h:32 * h + 1, :],
                                              y_ps[h][64:65, :])
                    nc.vector.reciprocal(den[:], den[:])
                    r1 = spool.tile([32, 512], F32, tag="rt1")
                    nc.gpsimd.tensor_copy(r1[0:1, :], den[32:33, :])
                    rdb = [spool1.tile([64, 512], F32, tag=f"rdb{h}",
                                       name=f"rdb{h}") for h in (0, 1)]
                    nc.gpsimd.partition_broadcast(rdb[0][:], den[0:1, :])
                    nc.gpsimd.partition_broadcast(rdb[1][:], r1[0:1, :])
                    for h in (0, 1):
                        nc.vector.tensor_mul(
                            y2_send[64 * h:64 * h + 64, 512 * tau:512 * (tau + 1)],
                            y_ps[h][0:64, :], rdb[h][:])
                    # broadcast y2 in halves as soon as computed (full
                    # broadcasts spread wire over all 16 engines)
                    if tau == 1:
                        comm_send("rsb", [(y2_send[:, 0:1024],
                                           rs_recv[:, ds(regs["xh_slot"], 1024)],
                                           None)], name=f"y2a{l}")
                    elif tau == 3:
                        comm_send("rsb", [(y2_send[:, 1024:2048],
                                           rs_recv[:, ds(regs["xh_slot2"], 1024)],
                                           None)], name=f"y2b{l}")
            if l == 0 and "y2" in tap_d:
                nc.sync.dma_start(tap_d["y2"], y2_send[:])

            # ---- wait all y2, then Wo directly from rs_recv (reg offset) ----
            wy = comm_wait("rsb", state["rs"]["rsb"], name=f"wY{l}")
            first_wo = None
            with tc.tile_pool(name=f"psw{l}", bufs=1, space="PSUM") as W:
                wo_ps = [W.tile([128, TLOC], F32, tag=f"wo{m}",
                                name=f"wo_ps{m}") for m in range(8)]
                for cp in range(8):
                    for m in range(8):
                        mm = nc.tensor.matmul(
                            wo_ps[m][:], wo_t[cp][:, m, :],
                            rsr[:, cp, ds(regs["y_slot_pe"], TLOC)],
                            start=(cp == 0), stop=(cp == 7))
                        if first_wo is None:
                            first_wo = mm
                            dep(mm, wy, "y2 arrival")
                for m in range(8):
                    nc.vector.tensor_add(x_resid[:, m, :], x_resid[:, m, :],
                                         wo_ps[m][:])
            if l == 0 and "xres" in tap_d:
                nc.sync.dma_start(tap_d["xres"], x_resid_f)

            # ---- norm + AG for MLP ----
            norm_to_xh_send()
            ag_send(f"agM{l}")
            state["ag_thr"].append(state["rs"]["xh"])

            # prefetch W3 during MLP
            w3_t = [w4pool.tile([128, 8, 128], BF16, tag=f"w3_{j}",
                                name=f"w3_t{l}_{j}") for j in range(4)]
            for j in range(4):
                nc.sync.dma_start(w3_t[j][:], w3_d[l, :, j])

            wm = comm_wait("xh", state["ag_thr"][-1], name=f"wM{l}")
            h_fm = act.tile([128, 4, T], BF16, tag="h_fm", name=f"h_fm{l}")
            # ---- MLP W1/W2 ----
            first_mlp = None
            with tc.tile_pool(name=f"psm{l}", bufs=2, space="PSUM") as M:
                for j in range(4):
                    for tau in range(4):
                        rhs = xh4[:, 2 * tau:2 * tau + 2, :, :]
                        a_ps = M.tile([128, 512], F32, tag="aps")
                        b_ps = M.tile([128, 512], F32, tag="bps")
                        for k in range(8):
                            mm = nc.tensor.matmul(
                                a_ps[:], w12_t[k][:, j, :], rhs[:, :, k, :],
                                start=(k == 0), stop=(k == 7))
                            if first_mlp is None:
                                first_mlp = mm
                                dep(mm, wm, "MLP AG arrival")
                        for k in range(8):
                            nc.tensor.matmul(
                                b_ps[:], w12_t[k][:, 4 + j, :], rhs[:, :, k, :],
                                start=(k == 0), stop=(k == 7))
                        sil = spool1.tile([128, 512], BF16, tag="sil")
                        nc.scalar.activation(sil[:], a_ps[:],
                                             mybir.ActivationFunctionType.Silu)
                        nc.vector.tensor_mul(h_fm[:, j, 512 * tau:512 * (tau + 1)],
                                             sil[:], b_ps[:])
            if l == 0 and "hfm" in tap_d:
                nc.sync.dma_start(tap_d["hfm"], h_fm[:].rearrange("p a b -> p (a b)"))

            # ---- W3 partials per slot (slot order), send as computed ----
            # prt buffers: tag d%3 x bufs=2 -> slot d+6 reuses slot d's
            # buffer; the pair-2 crit waits lsem for pair-0's sends, with
            # explicit edges for the copies that reuse those buffers.
            pair_lsem = []
            w6_wait = None
            with tc.tile_pool(name=f"ps3{l}", bufs=2, space="PSUM") as W3P:
                pend = []
                for d in range(8):
                    p3 = W3P.tile([128, 8, TLOC], F32, tag="w3p")
                    # m pairs (2m, 2m+1) share a 2KB PSUM bank and
                    # start_tensor_calc zeroes the WHOLE bank: only the
                    # even-m j=0 matmul may carry start=True.
                    for j in range(4):
                        for m in range(8):
                            nc.tensor.matmul(
                                p3[:, m, :], w3_t[j][:, m, :],
                                h_fm[:, j, ds(regs["tok_pe"][d], TLOC)],
                                start=(j == 0 and m % 2 == 0), stop=(j == 3),
                                skip_group_check=(m % 2 == 1))
                    prt = prtpool.tile([128, 8, TLOC], BF16, tag=f"prt{d % 3}")
                    ci = nc.vector.tensor_copy(prt[:], p3[:])
                    if d >= 6:
                        dep(ci, w6_wait, "prt buffer reused after send drained")
                    pend.append((prt[:].rearrange("p m t -> p (m t)"),
                                 rs_recv[:, ds(regs["xh_slot"], AGB)], d))
                    if d % 2 == 1:
                        gp = nc.gpsimd
                        with tc.tile_critical(name=f"rs{l}_{d // 2}"):
                            for src, dst, slot in pend:
                                rd = [(0, k) if k == slot else None
                                      for k in range(NCORES)]
                                gp.remote_dma_broadcast(
                                    out_ap=dst, in_ap=src,
                                    remote_sem=rsem["rsb"], local_sem=lsem,
                                    rdests=rd).then_inc(psem, 1)
                                state["preps"] += 1
                                state["lsem"] += 16
                                state["rs"]["rsb"] += 2
                            gp.wait_ge(psem, state["preps"])
                            gp.trigger_dma(count=len(pend))
                            if d == 5:
                                gp.wait_ge(lsem, pair_lsem[0])
                        if d == 5:
                            w6_wait = tc.prev_crit_insts[mybir.EngineType.Pool]
                        pair_lsem.append(state["lsem"])
                        pend = []

            # ---- wait RS arrivals; residual add tree on vector ----
            wr = comm_wait("rsb", state["rs"]["rsb"],
                           lsem_thr=state["lsem"], name=f"wR{l}")
            t_a = spool1.tile([128, AGB], BF16, tag="racc0")
            a0 = nc.vector.tensor_add(
                t_a[:], rs_recv[:, 0:AGB], rs_recv[:, AGB:2 * AGB])
            dep(a0, wr, "RS arrival")
            for s_ in range(2, 7):
                nc.vector.tensor_add(t_a[:], t_a[:],
                                     rs_recv[:, AGB * s_:AGB * (s_ + 1)])
            nc.vector.tensor_add(x_resid_f, x_resid_f, t_a[:])
            nc.vector.tensor_add(x_resid_f, x_resid_f,
                                 rs_recv[:, 7 * AGB:8 * AGB])
            if l == 0 and "rsrecv" in tap_d:
                nc.sync.dma_start(tap_d["rsrecv"], rs_recv)
            if l == 0 and "xres2" in tap_d:
                nc.sync.dma_start(tap_d["xres2"], x_resid_f)

            # prefetch next layer's wq/wkv before the next wait
            if l + 1 < L:
                wq_t = wpool.tile([128, 8, 128], BF16, tag="wq")
                nc.sync.dma_start(wq_t[:], wq_d[l + 1])
                wkv_t = wpool.tile([128, 8, 128], BF16, tag="wkv")
                nc.sync.dma_start(wkv_t[:], wkv_d[l + 1])

            # ---- norm + AG for next layer / final ----
            norm_to_xh_send()
            ag_send(f"agN{l}")
            state["lsem_prev"] = state["lsem"]
            state["ag_thr"].append(state["rs"]["xh"])

        # ================= lm head =================
        layer_ctx.close()
        with tc.tile_pool(name="embp", bufs=8) as embp, \
             tc.tile_pool(name="outp", bufs=4) as outp, \
             tc.tile_pool(name="pslm", bufs=1, space="PSUM") as LM:
            embt = [embp.tile([128, VLOC], BF16, tag="embt", name="embt_t")
                    for _ in range(8)]
            for k in range(8):
                nc.sync.dma_start(embt[k][:], embt_d[:, k, :])
            wf = comm_wait("xh", state["ag_thr"][-1],
                           lsem_thr=state["lsem"], name="wF")
            first_lm = None
            lm_ps = [LM.tile([128, 500], F32, tag=f"lm{v}", name=f"lm_ps{v}")
                     for v in range(8)]
            for i in range(16):
                cpr, half = i // 2, i % 2
                for k in range(8):
                    lh = xh[:, cpr * AGB + k * TLOC + half * 128:
                            cpr * AGB + k * TLOC + half * 128 + 128]
                    for v in range(8):
                        mm = nc.tensor.matmul(lm_ps[v][:], lh,
                                              embt[k][:, 500 * v:500 * (v + 1)],
                                              start=(k == 0), stop=(k == 7))
                        if first_lm is None:
                            first_lm = mm
                            dep(mm, wf, "final AG arrival")
                for v in range(8):
                    o = outp.tile([128, 500], F32, tag="o")
                    if v % 2 == 0:
                        nc.vector.tensor_copy(o[:], lm_ps[v][:])
                    else:
                        nc.scalar.copy(o[:], lm_ps[v][:])
                    nc.sync.dma_start(
                        logits_d[128 * i:128 * (i + 1), 500 * v:500 * (v + 1)], o[:])

    nc.compile()
    return nc


# ======================= host side =======================

def prep_inputs(inputs):
    bf = ml_dtypes.bfloat16
    tokens = np.asarray(inputs["tokens"])
    emb = np.asarray(inputs["emb"], np.float32)
    anw = np.asarray(inputs["attn_norm_w"], np.float32)
    Wq = np.asarray(inputs["Wq"], np.float32)
    Wk = np.asarray(inputs["Wk"], np.float32)
    Wv = np.asarray(inputs["Wv"], np.float32)
    Wo = np.asarray(inputs["Wo"], np.float32)
    ffw = np.asarray(inputs["ff_norm_w"], np.float32)
    W1 = np.asarray(inputs["W1"], np.float32)
    W2 = np.asarray(inputs["W2"], np.float32)
    W3 = np.asarray(inputs["W3"], np.float32)
    nfw = np.asarray(inputs["norm_f_w"], np.float32)

    Wq_s = Wq * anw[:, None, :]
    Wk_s = Wk * anw[:, None, :] / 8.0
    Wv_s = Wv * anw[:, None, :]
    W1_s = W1 * ffw[:, None, :]
    W2_s = W2 * ffw[:, None, :]
    emb_s = emb * nfw[None, :]

    pos = np.arange(T, dtype=np.float64)
    inv = 1.0 / (10000.0 ** (np.arange(32, dtype=np.float64) / 32.0))
    ang = pos[:, None] * inv[None, :]
    cos_fm = np.cos(ang).T.astype(np.float32)    # [32, T]
    sin_fm = np.sin(ang).T.astype(np.float32)
    rope = np.concatenate([cos_fm, sin_fm, cos_fm, sin_fm], 0).astype(bf)

    p_ = np.arange(128)[:, None]
    g_ = np.arange(896)[None, :] - 384
    dmask = np.where(p_ > g_, np.float32(0.0), np.float32(1.0)).astype(bf)

    toks = tokens.reshape(-1)
    in_maps = []
    for c in range(NCORES):
        wq_in = rearrange(Wq_s[:, 128 * c:128 * (c + 1), :],
                          "l m (k p) -> l p k m", p=128).astype(bf)
        kp = rearrange(Wk_s[:, 64 * c:64 * (c + 1), :],
                       "l m (k p) -> l p k m", p=128)
        vp = rearrange(Wv_s[:, 64 * c:64 * (c + 1), :],
                       "l m (k p) -> l p k m", p=128)
        wkv_in = np.concatenate([kp, vp], -1).astype(bf)
        wo_in = rearrange(Wo, "l (m mm) (cp p) -> l p cp m mm",
                          mm=128, p=128).astype(bf)
        w1p = rearrange(W1_s[:, 512 * c:512 * (c + 1), :],
                        "l (j jj) (k p) -> l p k j jj", jj=128, p=128)
        w2p = rearrange(W2_s[:, 512 * c:512 * (c + 1), :],
                        "l (j jj) (k p) -> l p k j jj", jj=128, p=128)
        w12_in = np.concatenate([w1p, w2p], 3).astype(bf)
        w3_in = rearrange(W3[:, :, 512 * c:512 * (c + 1)],
                          "l (m mm) (j p) -> l p j m mm", mm=128, p=128).astype(bf)
        embt_in = rearrange(emb_s[VLOC * c:VLOC * (c + 1), :],
                            "vv (k p) -> p k vv", p=128).astype(bf)
        x0 = emb[toks[TLOC * c:TLOC * (c + 1)]]
        x0_in = rearrange(x0, "(i p) cc -> i p cc", p=128).astype(np.float32)
        info = np.zeros((1, 32), np.int32)
        info[0, 0] = c * AGB
        info[0, 1] = c * TLOC
        info[0, 2] = c * AGB + 1024
        for d in range(8):
            dlog = PHYS_INV[PHYS[c] ^ d]
            info[0, 8 + d] = dlog * TLOC
        in_maps.append({
            "wq": wq_in, "wkv": wkv_in, "wo": wo_in, "w12": w12_in,
            "w3": w3_in, "embt": embt_in, "x0": x0_in, "rope": rope,
            "dmask": dmask, "coreinfo": info,
        })
    return in_maps


def assemble(results):
    return np.concatenate([r["logits"] for r in results], axis=1)[None]


# ======================= harness entry point =======================

_CACHE = {}


def kernel(**inputs):
    """Full-model entry: takes unsharded inputs, returns [1, T, V] logits."""
    from concourse.bass_utils import run_bass_kernel_spmd
    if "nc" not in _CACHE:
        _CACHE["nc"] = build_nc()
    nc = _CACHE["nc"]
    in_maps = prep_inputs(inputs)
    res = run_bass_kernel_spmd(nc, in_maps, core_ids=list(range(NCORES)))
    return assemble(res.results).astype(np.float32)



# revision 17
# speedup vs baseline: 1.0417x; 1.0417x over previous
"""DeepSeek-style LM on 8 TRN2 cores — tensor-parallel Bass/Tile kernel.

Sharding: 2 Q heads + 1 KV head per core, HFF/8 per core, V/8 per core,
token-block/8 per core for residual+norm. Cross-core comm via
remote_dma_broadcast (SBUF->SBUF). Activations feature-major [C_part, T].

v2: pipelined communication — sends fire as soon as chunks are computed
(y2 per tau-pair, W3 reduce-scatter per slot in slot order), arrival
waits deferred to just before the first consumer. No credit handshake
(global round order transitively protects buffer reuse).
"""

import numpy as np
import ml_dtypes
from contextlib import ExitStack
from einops import rearrange

import concourse.bass as bass
import concourse.tile as tile
from concourse import bacc, mybir
from concourse.bass import ds
from concourse.masks import make_identity

F32 = mybir.dt.float32
BF16 = mybir.dt.bfloat16
I32 = mybir.dt.int32

NCORES = 8
T, C, DH, L, V, HFF = 2048, 1024, 64, 4, 32000, 4096
TLOC = T // NCORES          # 256
VLOC = V // NCORES          # 4000
EPS = 1e-5
AGB = 8 * TLOC              # 2048 elems: one sender's block in xh/rs_recv

# jax device index -> physical NeuronCore (tpb) index on this chip.
# remote_dma relative addressing XORs PHYSICAL tpb ids, so per-slot logical
# destinations are P^-1(P(c) ^ slot).
PHYS = [4, 5, 6, 7, 2, 3, 0, 1]
PHYS_INV = [PHYS.index(i) for i in range(8)]


def build_nc(taps=()):
    nc = bacc.Bacc("TRN2", target_bir_lowering=False, debug=False,
                   num_devices=NCORES)

    # ---------------- DRAM I/O ----------------
    dt = nc.dram_tensor
    wq_d = dt("wq", [L, 128, 8, 128], BF16, kind="ExternalInput").ap()
    wkv_d = dt("wkv", [L, 128, 8, 128], BF16, kind="ExternalInput").ap()
    wo_d = dt("wo", [L, 128, 8, 8, 128], BF16, kind="ExternalInput").ap()
    w12_d = dt("w12", [L, 128, 8, 8, 128], BF16, kind="ExternalInput").ap()
    w3_d = dt("w3", [L, 128, 4, 8, 128], BF16, kind="ExternalInput").ap()
    embt_d = dt("embt", [128, 8, VLOC], BF16, kind="ExternalInput").ap()
    x0_d = dt("x0", [2, 128, C], F32, kind="ExternalInput").ap()
    rope_d = dt("rope", [128, T], BF16, kind="ExternalInput").ap()
    dmask_d = dt("dmask", [128, 896], BF16, kind="ExternalInput").ap()
    info_d = dt("coreinfo", [1, 32], I32, kind="ExternalInput").ap()
    logits_d = dt("logits", [T, VLOC], F32, kind="ExternalOutput").ap()
    tap_d = {}
    for t_ in taps:
        shp = {"xh": [128, NCORES * AGB], "qfm": [128, T], "k2": [128, T],
               "vaug": [128, 16 * 65], "y2": [128, T],
               "xres": [128, 8 * TLOC], "hfm": [128, 4 * T],
               "rsrecv": [128, NCORES * AGB],
               "xres2": [128, 8 * TLOC]}[t_]
        dtp = F32 if t_ in ('xres', 'xres2') else BF16
        tap_d[t_] = dt("tap_" + t_, shp, dtp, kind="ExternalOutput").ap()

    # ---------------- static SBUF (fixed addrs for remote writes) -------
    xh = nc.alloc_sbuf_tensor("xh", [128, NCORES * AGB], BF16).ap()
    rs_recv = nc.alloc_sbuf_tensor("rs_recv", [128, NCORES * AGB], BF16).ap()
    info_sb = nc.alloc_sbuf_tensor("info_sb", [1, 32], I32).ap()

    xh4 = xh.rearrange("p (c k t) -> p c k t", c=NCORES, k=8)
    rsr = rs_recv.rearrange("p (c t) -> p c t", c=NCORES)

    # ---------------- semaphores ----------------
    sem = nc.alloc_semaphore
    lsem = sem("lsem")
    psem = sem("psem")
    dmas = sem("dmas")
    w3s = sem("w3sem")   # local-completion sem used ONLY by pair-0 W3 sends
    rsem = {k: sem(f"rsem_{k}") for k in ("xh", "rsb")}
    RD_ALL = [(0, k) for k in range(NCORES)]

    # cumulative counters (python-side bookkeeping of semaphore targets)
    state = dict(preps=0, lsem=0, w3s=0, rs={"xh": 0, "rsb": 0})

    def comm_send(kind, sends, name=""):
        """Issue remote broadcasts; no arrival wait. sends: [(src, dst, slot)].

        no_gpsimd_drain: the post-crit gpsimd drain would block until the
        remote DMA wire transfer completes (12-31us each). Buffer-reuse
        safety comes from explicit lsem waits at wA/wY/wR/wF instead."""
        gp = nc.gpsimd
        with tc.tile_critical(name=name, no_gpsimd_drain=True):
            for src, dst, slot in sends:
                rd = RD_ALL if slot is None else \
                    [(0, k) if k == slot else None for k in range(NCORES)]
                gp.remote_dma_broadcast(out_ap=dst, in_ap=src,
                                        remote_sem=rsem[kind], local_sem=lsem,
                                        rdests=rd).then_inc(psem, 1)
                state["preps"] += 1
                state["lsem"] += 16
                state["rs"][kind] += 2 if slot is not None else 16
            gp.wait_ge(psem, state["preps"])
            gp.trigger_dma(count=len(sends))

    def comm_wait(kind, target, lsem_thr=None, name=""):
        """Wait for arrivals (and optionally local send completion).
        Returns the crit's post-crit instruction for add_dep_helper edges
        (the scheduler treats crit bodies as opaque, so edges must target
        the post-crit barrier, not the inner wait)."""
        gp = nc.gpsimd
        with tc.tile_critical(name=name, no_gpsimd_drain=True):
            gp.wait_ge(rsem[kind], target)
            if lsem_thr is not None:
                gp.wait_ge(lsem, lsem_thr)
        return tc.prev_crit_insts[mybir.EngineType.Pool]

    def dep(consumer, waiter, why="comm arrival"):
        tile.add_dep_helper(consumer.ins, waiter, True, why)

    with tile.TileContext(nc) as tc, ExitStack() as ctx:
        # ---------- pools ----------
        sing = ctx.enter_context(tc.tile_pool(name="sing", bufs=1))
        spool = ctx.enter_context(tc.tile_pool(name="spool", bufs=1))
        spool1 = ctx.enter_context(tc.tile_pool(name="spool1", bufs=1))
        layer_ctx = ExitStack()
        act = layer_ctx.enter_context(tc.tile_pool(name="act", bufs=1))
        wpool = layer_ctx.enter_context(tc.tile_pool(name="wpool", bufs=2))
        wopool = layer_ctx.enter_context(tc.tile_pool(name="wopool", bufs=2))
        w8pool = layer_ctx.enter_context(tc.tile_pool(name="w8pool", bufs=1))
        w4pool = layer_ctx.enter_context(tc.tile_pool(name="w4pool", bufs=1))
        ppool = layer_ctx.enter_context(tc.tile_pool(name="ppool", bufs=2))
        prtpool = layer_ctx.enter_context(tc.tile_pool(name="prtpool", bufs=1))

        # ---------- constants ----------
        ident = sing.tile([128, 128], F32)
        make_identity(nc, ident)
        ident_bf = sing.tile([64, 64], BF16)
        nc.vector.tensor_copy(ident_bf[:], ident[0:64, 0:64])
        ones_sb = sing.tile([128, 128], BF16)
        nc.vector.memset(ones_sb, 1.0)
        rope_sb = sing.tile([128, T], BF16)
        nc.sync.dma_start(rope_sb[:], rope_d)
        eps_sb = sing.tile([128, 1], F32)
        nc.vector.memset(eps_sb, EPS)
        dmask_sb = sing.tile([128, 896], BF16)
        nc.sync.dma_start(dmask_sb[:], dmask_d)

        # persistent activations
        x_resid = sing.tile([128, 8, TLOC], F32)
        xh_send = sing.tile([128, 8, TLOC], BF16)
        xh_send_f = xh_send[:].rearrange("p k t -> p (k t)")
        x_resid_f = x_resid[:].rearrange("p k t -> p (k t)")

        # ---------- registers from coreinfo ----------
        regs = {}
        with tc.tile_critical():
            nc.gpsimd.dma_start(info_sb, info_d).then_inc(dmas, 16)
            nc.gpsimd.wait_ge(dmas, 16)

            def ld(eng, idx, mx):
                r = eng.alloc_register(f"r{idx}")
                eng.reg_load(r, info_sb[0:1, idx:idx + 1])
                return eng.snap(r, donate=True, min_val=0, max_val=mx)
            regs["xh_slot"] = ld(nc.gpsimd, 0, (NCORES - 1) * AGB)
            regs["y_slot"] = ld(nc.gpsimd, 1, (NCORES - 1) * TLOC)
            regs["xh_slot2"] = ld(nc.gpsimd, 2, (NCORES - 1) * AGB + 1024)
            regs["tok"] = [ld(nc.gpsimd, 8 + d, (NCORES - 1) * TLOC)
                           for d in range(8)]
            # y2 per-dest landing offsets: own_block_base + dest's y_slot
            regs["y2dst"] = [ld(nc.gpsimd, 16 + d,
                                (NCORES - 1) * AGB + (NCORES - 1) * TLOC)
                             for d in range(8)]
            # PE-engine copies for reg-offset moving APs (regs are per-engine)
            nc.tensor.wait_ge(dmas, 16)
            regs["y_slot_pe"] = ld(nc.tensor, 1, (NCORES - 1) * TLOC)
            regs["tok_pe"] = [ld(nc.tensor, 8 + d, (NCORES - 1) * TLOC)
                              for d in range(8)]

        # ---------- helpers ----------
        def norm_to_xh_send():
            """xh_send = rmsnorm(x_resid); first writer is a vector mul that
            is ordered behind the preceding comm_wait via vector-queue FIFO."""
            with tc.tile_pool(name="psn", bufs=1, space="PSUM") as psn:
                ns = psn.tile([128, TLOC], F32)
                for k in range(8):
                    sq = spool1.tile([128, TLOC], BF16, tag="sq")
                    nc.vector.tensor_mul(sq[:], x_resid[:, k, :], x_resid[:, k, :])
                    nc.tensor.matmul(ns[:], ones_sb[:], sq[:],
                                     start=(k == 0), stop=(k == 7))
                rms = spool1.tile([128, TLOC], F32, tag="rms")
                nc.scalar.activation(rms[:], ns[:],
                                     mybir.ActivationFunctionType.Sqrt,
                                     bias=eps_sb[:], scale=1.0 / C)
                rin = spool1.tile([128, TLOC], F32, tag="rin")
                nc.vector.reciprocal(rin[:], rms[:])
                for k in range(8):
                    nc.vector.tensor_mul(xh_send[:, k, :], x_resid[:, k, :], rin[:])
                    if k == 3:
                        comm_send("xh", [(xh_send_f[:, 0:1024],
                                          xh[:, ds(regs["xh_slot"], 1024)],
                                          None)], name="agA")

        def ag_send(name):
            comm_send("xh", [(xh_send_f[:, 1024:2048],
                              xh[:, ds(regs["xh_slot2"], 1024)], None)],
                      name=name)

        # ---------- x0 init ----------
        with tc.tile_pool(name="x0p", bufs=2) as x0p:
            with tc.tile_pool(name="ps0", bufs=2, space="PSUM") as ps0:
                for i in range(2):
                    x0_sb = x0p.tile([128, C], F32, tag="x0", name="x0_sb")
                    nc.sync.dma_start(x0_sb[:], x0_d[i])
                    for k in range(8):
                        tp = ps0.tile([128, 128], F32)
                        nc.tensor.transpose(tp[:], x0_sb[:, 128 * k:128 * (k + 1)],
                                            ident[:])
                        nc.vector.tensor_copy(x_resid[:, k, 128 * i:128 * (i + 1)],
                                              tp[:])
        norm_to_xh_send()
        ag_send("ag0")   # round 0: xh <- layer-0 attn input
        state.setdefault("ag_thr", []).append(state["rs"]["xh"])
        # lsem count covering the attn-AG sends: wY waits this before the
        # post-Wo norm rewrites xh_send (post-crit drains no longer cover it)
        state["lsem_attn"] = state["lsem"]

        def rope_apply(out_fm, ps, base, tau):
            """rotate-half on psum rows [base:base+64] -> out_fm bf16."""
            sl = slice(512 * tau, 512 * (tau + 1))
            cos = rope_sb[base:base + 32, sl]
            sin = rope_sb[base + 32:base + 64, sl]
            x1 = ps[base:base + 32, :]
            x2 = ps[base + 32:base + 64, :]
            t1 = spool.tile([32, 512], F32, tag="rt1")
            t2 = spool.tile([32, 512], F32, tag="rt2")
            nc.vector.tensor_mul(t1[:], x1, cos)
            nc.vector.tensor_mul(t2[:], x2, sin)
            nc.vector.tensor_sub(out_fm[base:base + 32, sl], t1[:], t2[:])
            nc.vector.tensor_mul(t1[:], x1, sin)
            nc.vector.tensor_mul(t2[:], x2, cos)
            nc.vector.tensor_add(out_fm[base + 32:base + 64, sl], t1[:], t2[:])

        # wq/wkv for layer 0 prefetched before the first wait
        wq_t = wpool.tile([128, 8, 128], BF16, tag="wq")
        nc.sync.dma_start(wq_t[:], wq_d[0])
        wkv_t = wpool.tile([128, 8, 128], BF16, tag="wkv")
        nc.sync.dma_start(wkv_t[:], wkv_d[0])

        # ================= layers =================
        for l in range(L):
            # ---- wait for this layer's attn all-gather ----
            wa = comm_wait("xh", state["ag_thr"][-1],
                           lsem_thr=state.get("lsem_prev"), name=f"wA{l}")
            q_fm = act.tile([128, T], BF16, tag="q_fm", name=f"q_fm{l}")
            k2_fm = act.tile([128, T], BF16, tag="k2_fm", name=f"k2_fm{l}")
            v_aug = act.tile([128, 16, 65], BF16, tag="v_aug", name=f"v_aug{l}")
            y2_send = act.tile([128, T], BF16, tag="y2_send", name=f"y2_send{l}")

            # prefetch Wo during QKV/attention (sync queue, before any crit
            # that could trap it behind the y wait; bufs=2 paces the stream)
            wo_t = []
            for cp in range(8):
                w_ = wopool.tile([128, 8, 128], BF16, tag="wo",
                                 name=f"wo_t{l}_{cp}")
                nc.sync.dma_start(w_[:], wo_d[l, :, cp])
                wo_t.append(w_)

            # ---- QKV matmuls back-to-back; v transposes deferred so the PE
            # stream is not interleaved with vector-dependent work ----
            first_qkv = None
            v_stf = act.tile([64, T], BF16, tag="v_stf", name=f"v_stf{l}")
            with tc.tile_pool(name=f"psq{l}", bufs=2, space="PSUM") as P, \
                 tc.tile_pool(name=f"psk{l}", bufs=4, space="PSUM") as PK, \
                 tc.tile_pool(name=f"psv{l}", bufs=2, space="PSUM") as PV:
                for tau in range(4):
                    rhs = xh4[:, 2 * tau:2 * tau + 2, :, :]
                    q_ps = P.tile([128, 512], F32, tag="q")
                    for k in range(8):
                        mm = nc.tensor.matmul(
                            q_ps[:], wq_t[:, k, :], rhs[:, :, k, :],
                            start=(k == 0), stop=(k == 7))
                        if first_qkv is None:
                            first_qkv = mm
                            dep(mm, wa, "attn AG arrival")
                    rope_apply(q_fm, q_ps, 0, tau)
                    rope_apply(q_fm, q_ps, 64, tau)
                    kv_ps = PK.tile([128, 512], F32, tag="k")
                    for k in range(8):
                        nc.tensor.matmul(
                            kv_ps[:], wkv_t[:, k, :], rhs[:, :, k, :],
                            start=(k == 0), stop=(k == 7))
                    rope_apply(k2_fm, kv_ps, 0, tau)
                    # duplicate k rows for head 1 (scalar engine is idle here)
                    nc.scalar.copy(k2_fm[64:128, 512 * tau:512 * (tau + 1)],
                                   k2_fm[0:64, 512 * tau:512 * (tau + 1)])
                    nc.vector.tensor_copy(v_stf[:, 512 * tau:512 * (tau + 1)],
                                          kv_ps[64:128, :])
                for i in range(16):
                    vt = PV.tile([128, 64], BF16, tag="vtr")
                    nc.tensor.transpose(vt[:], v_stf[:, 128 * i:128 * (i + 1)],
                                        ident_bf[0:64, 0:64])
                    nc.vector.tensor_copy(v_aug[:, i, 0:64], vt[:])
                nc.vector.memset(v_aug[:, :, 64:65], 1.0)
            if l == 0 and "qfm" in tap_d:
                nc.sync.dma_start(tap_d["qfm"], q_fm[:])
            if l == 0 and "k2" in tap_d:
                nc.sync.dma_start(tap_d["k2"], k2_fm[:])
            if l == 0 and "vaug" in tap_d:
                nc.sync.dma_start(tap_d["vaug"],
                                  v_aug[:].rearrange("p a b -> p (a b)"))

            # prefetch W1/W2 during attention
            w12_t = [w8pool.tile([128, 8, 128], BF16, tag=f"w12_{k}",
                                 name=f"w12_t{l}_{k}") for k in range(8)]
            for k in range(8):
                nc.sync.dma_start(w12_t[k][:], w12_d[l, :, k])

            # ---- scores + softmax + AV; y2 broadcast per tau-pair ----
            with tc.tile_pool(name=f"psa{l}", bufs=2, space="PSUM") as B:
                for tau in range(4):
                    y_ps = [B.tile([65, 512], F32, tag=f"y{h}",
                                   name=f"y_ps{h}") for h in (0, 1)]
                    na = 4 * tau + 4
                    for a in range(na):
                        pts = []
                        for h in (0, 1):
                            s_ps = B.tile([128, 512], F32, tag=f"s{h}")
                            nc.tensor.matmul(
                                s_ps[:],
                                k2_fm[64 * h:64 * h + 64, 128 * a:128 * (a + 1)],
                                q_fm[64 * h:64 * h + 64, 512 * tau:512 * (tau + 1)],
                                start=True, stop=True)
                            p_t = ppool.tile([128, 512], BF16, tag=f"pT{h}")
                            nc.scalar.activation(p_t[:], s_ps[:],
                                                 mybir.ActivationFunctionType.Exp)
                            if a >= 4 * tau:
                                r_ = a - 4 * tau
                                nc.vector.tensor_mul(
                                    p_t[:], p_t[:],
                                    dmask_sb[:, 384 - 128 * r_:896 - 128 * r_])
                            pts.append(p_t)
                        for h in (0, 1):
                            nc.tensor.matmul(y_ps[h][:], v_aug[:, a, :], pts[h][:],
                                             start=(a == 0), stop=(a == na - 1))
                    # softmax denominators: batch both heads in one reciprocal
                    # (partition_broadcast needs base-0 in/out; gpsimd copies
                    # shift rows to base 0 first)
                    den = spool1.tile([33, 512], F32, tag="den")
                    for h in (0, 1):
                        nc.vector.tensor_copy(den[32 * h:32 * h + 1, :],
                                              y_ps[h][64:65, :])
                    nc.vector.reciprocal(den[:], den[:])
                    r1 = spool.tile([32, 512], F32, tag="rt1")
                    nc.gpsimd.tensor_copy(r1[0:1, :], den[32:33, :])
                    rdb = [spool1.tile([64, 512], F32, tag=f"rdb{h}",
                                       name=f"rdb{h}") for h in (0, 1)]
                    nc.gpsimd.partition_broadcast(rdb[0][:], den[0:1, :])
                    nc.gpsimd.partition_broadcast(rdb[1][:], r1[0:1, :])
                    for h in (0, 1):
                        nc.vector.tensor_mul(
                            y2_send[64 * h:64 * h + 64, 512 * tau:512 * (tau + 1)],
                            y_ps[h][0:64, :], rdb[h][:])

            if l == 0 and "y2" in tap_d:
                nc.sync.dma_start(tap_d["y2"], y2_send[:])

            # ---- per-dest y2 reduce-scatter: dest d only needs its own
            # 256-token slice (wire 0.5MB vs 3.5MB broadcast). Slice and dst
            # offsets are register-indexed (slot->token-block map is per-core).
            gp = nc.gpsimd
            with tc.tile_critical(name=f"y2s{l}", no_gpsimd_drain=True):
                for d in range(8):
                    rd = [(0, k) if k == d else None for k in range(NCORES)]
                    gp.remote_dma_broadcast(
                        out_ap=rs_recv[:, ds(regs["y2dst"][d], TLOC)],
                        in_ap=y2_send[:, ds(regs["tok"][d], TLOC)],
                        remote_sem=rsem["rsb"], local_sem=lsem,
                        rdests=rd).then_inc(psem, 1)
                    state["preps"] += 1
                    state["lsem"] += 16
                    state["rs"]["rsb"] += 2
                gp.wait_ge(psem, state["preps"])
                gp.trigger_dma(count=8)

            # ---- wait all y2, then Wo directly from rs_recv (reg offset) ----
            # lsem_thr must be the FULL issued count: lsem increments arrive
            # out of order across engines, so a partial-count threshold can be
            # satisfied by later sends' lanes while earlier lanes still read.
            wy = comm_wait("rsb", state["rs"]["rsb"],
                           lsem_thr=state["lsem"], name=f"wY{l}")
            first_wo = None
            with tc.tile_pool(name=f"psw{l}", bufs=1, space="PSUM") as W:
                wo_ps = [W.tile([128, TLOC], F32, tag=f"wo{m}",
                                name=f"wo_ps{m}") for m in range(8)]
                for cp in range(8):
                    for m in range(8):
                        mm = nc.tensor.matmul(
                            wo_ps[m][:], wo_t[cp][:, m, :],
                            rsr[:, cp, ds(regs["y_slot_pe"], TLOC)],
                            start=(cp == 0), stop=(cp == 7))
                        if first_wo is None:
                            first_wo = mm
                            dep(mm, wy, "y2 arrival")
                for m in range(8):
                    nc.vector.tensor_add(x_resid[:, m, :], x_resid[:, m, :],
                                         wo_ps[m][:])
            if l == 0 and "xres" in tap_d:
                nc.sync.dma_start(tap_d["xres"], x_resid_f)

            # ---- norm + AG for MLP ----
            norm_to_xh_send()
            ag_send(f"agM{l}")
            state["ag_thr"].append(state["rs"]["xh"])

            # prefetch W3 during MLP
            w3_t = [w4pool.tile([128, 8, 128], BF16, tag=f"w3_{j}",
                                name=f"w3_t{l}_{j}") for j in range(4)]
            for j in range(4):
                nc.sync.dma_start(w3_t[j][:], w3_d[l, :, j])

            wm = comm_wait("xh", state["ag_thr"][-1], name=f"wM{l}")
            h_fm = act.tile([128, 4, T], BF16, tag="h_fm", name=f"h_fm{l}")
            # ---- MLP W1/W2 ----
            first_mlp = None
            with tc.tile_pool(name=f"psm{l}", bufs=2, space="PSUM") as M:
                for j in range(4):
                    for tau in range(4):
                        rhs = xh4[:, 2 * tau:2 * tau + 2, :, :]
                        a_ps = M.tile([128, 512], F32, tag="aps")
                        b_ps = M.tile([128, 512], F32, tag="bps")
                        for k in range(8):
                            mm = nc.tensor.matmul(
                                a_ps[:], w12_t[k][:, j, :], rhs[:, :, k, :],
                                start=(k == 0), stop=(k == 7))
                            if first_mlp is None:
                                first_mlp = mm
                                dep(mm, wm, "MLP AG arrival")
                        for k in range(8):
                            nc.tensor.matmul(
                                b_ps[:], w12_t[k][:, 4 + j, :], rhs[:, :, k, :],
                                start=(k == 0), stop=(k == 7))
                        sil = spool1.tile([128, 512], BF16, tag="sil")
                        nc.scalar.activation(sil[:], a_ps[:],
                                             mybir.ActivationFunctionType.Silu)
                        nc.vector.tensor_mul(h_fm[:, j, 512 * tau:512 * (tau + 1)],
                                             sil[:], b_ps[:])
            if l == 0 and "hfm" in tap_d:
                nc.sync.dma_start(tap_d["hfm"], h_fm[:].rearrange("p a b -> p (a b)"))

            # ---- W3 partials per slot (slot order), send as computed ----
            # 6 prt buffers; d=6,7 reuse pair-0's buffers. Pair-0 sends use
            # the dedicated w3s local sem, so the reuse wait is a FULL-count
            # threshold on w3s (race-free); all other sends use lsem and are
            # covered by wR's full-count lsem threshold.
            w6_wait = None
            with tc.tile_pool(name=f"ps3{l}", bufs=2, space="PSUM") as W3P:
                pend = []
                for d in range(8):
                    p3 = W3P.tile([128, 8, TLOC], F32, tag="w3p")
                    # m pairs (2m, 2m+1) share a 2KB PSUM bank and
                    # start_tensor_calc zeroes the WHOLE bank: only the
                    # even-m j=0 matmul may carry start=True.
                    for j in range(4):
                        for m in range(8):
                            nc.tensor.matmul(
                                p3[:, m, :], w3_t[j][:, m, :],
                                h_fm[:, j, ds(regs["tok_pe"][d], TLOC)],
                                start=(j == 0 and m % 2 == 0), stop=(j == 3),
                                skip_group_check=(m % 2 == 1))
                    prt = prtpool.tile([128, 8, TLOC], BF16, tag=f"prt{d % 6}")
                    ci = nc.vector.tensor_copy(prt[:], p3[:])
                    if d >= 6:
                        dep(ci, w6_wait, "prt buffer reused after send drained")
                    pend.append((prt[:].rearrange("p m t -> p (m t)"),
                                 rs_recv[:, ds(regs["xh_slot"], AGB)], d))
                    if d % 2 == 1:
                        pair0 = (d == 1)
                        lls = w3s if pair0 else lsem
                        gp = nc.gpsimd
                        with tc.tile_critical(name=f"rs{l}_{d // 2}",
                                              no_gpsimd_drain=True):
                            for src, dst, slot in pend:
                                rd = [(0, k) if k == slot else None
                                      for k in range(NCORES)]
                                gp.remote_dma_broadcast(
                                    out_ap=dst, in_ap=src,
                                    remote_sem=rsem["rsb"], local_sem=lls,
                                    rdests=rd).then_inc(psem, 1)
                                state["preps"] += 1
                                state["w3s" if pair0 else "lsem"] += 16
                                state["rs"]["rsb"] += 2
                            gp.wait_ge(psem, state["preps"])
                            gp.trigger_dma(count=len(pend))
                            if d == 5:
                                gp.wait_ge(w3s, state["w3s"])
                        if d == 5:
                            w6_wait = tc.prev_crit_insts[mybir.EngineType.Pool]
                        pend = []

            # ---- wait RS arrivals; residual add tree on vector ----
            wr = comm_wait("rsb", state["rs"]["rsb"],
                           lsem_thr=state["lsem"], name=f"wR{l}")
            t_a = spool1.tile([128, AGB], BF16, tag="racc0")
            a0 = nc.vector.tensor_add(
                t_a[:], rs_recv[:, 0:AGB], rs_recv[:, AGB:2 * AGB])
            dep(a0, wr, "RS arrival")
            for s_ in range(2, 7):
                nc.vector.tensor_add(t_a[:], t_a[:],
                                     rs_recv[:, AGB * s_:AGB * (s_ + 1)])
            nc.vector.tensor_add(x_resid_f, x_resid_f, t_a[:])
            nc.vector.tensor_add(x_resid_f, x_resid_f,
                                 rs_recv[:, 7 * AGB:8 * AGB])
            if l == 0 and "rsrecv" in tap_d:
                nc.sync.dma_start(tap_d["rsrecv"], rs_recv)
            if l == 0 and "xres2" in tap_d:
                nc.sync.dma_start(tap_d["xres2"], x_resid_f)

            # prefetch next layer's wq/wkv before the next wait
            if l + 1 < L:
                wq_t = wpool.tile([128, 8, 128], BF16, tag="wq")
                nc.sync.dma_start(wq_t[:], wq_d[l + 1])
                wkv_t = wpool.tile([128, 8, 128], BF16, tag="wkv")
                nc.sync.dma_start(wkv_t[:], wkv_d[l + 1])

            # ---- norm + AG for next layer / final ----
            norm_to_xh_send()
            ag_send(f"agN{l}")
            state["lsem_prev"] = state["lsem"]
            state["lsem_attn"] = state["lsem"]
            state["ag_thr"].append(state["rs"]["xh"])

        # ================= lm head =================
        layer_ctx.close()
        with tc.tile_pool(name="embp", bufs=8) as embp, \
             tc.tile_pool(name="outp", bufs=4) as outp, \
             tc.tile_pool(name="pslm", bufs=2, space="PSUM") as LM:
            embt = [embp.tile([128, VLOC], BF16, tag="embt", name="embt_t")
                    for _ in range(8)]
            for k in range(8):
                nc.sync.dma_start(embt[k][:], embt_d[:, k, :])
            wf = comm_wait("xh", state["ag_thr"][-1],
                           lsem_thr=state["lsem"], name="wF")
            first_lm = None
            # 4 vocab tiles x bufs=2: set s's copies overlap set s+1's matmuls
            for i in range(16):
                cpr, half = i // 2, i % 2
                for s in range(2):
                    lm_ps = [LM.tile([128, 500], F32, tag=f"lm{v}",
                                     name=f"lm_ps{v}") for v in range(4)]
                    for k in range(8):
                        lh = xh[:, cpr * AGB + k * TLOC + half * 128:
                                cpr * AGB + k * TLOC + half * 128 + 128]
                        for v in range(4):
                            vv = 4 * s + v
                            mm = nc.tensor.matmul(
                                lm_ps[v][:], lh,
                                embt[k][:, 500 * vv:500 * (vv + 1)],
                                start=(k == 0), stop=(k == 7))
                            if first_lm is None:
                                first_lm = mm
                                dep(mm, wf, "final AG arrival")
                    for v in range(4):
                        vv = 4 * s + v
                        o = outp.tile([128, 500], F32, tag="o")
                        if v % 2 == 0:
                            nc.vector.tensor_copy(o[:], lm_ps[v][:])
                        else:
                            nc.scalar.copy(o[:], lm_ps[v][:])
                        nc.sync.dma_start(
                            logits_d[128 * i:128 * (i + 1),
                                     500 * vv:500 * (vv + 1)], o[:])

    nc.compile()
    return nc


# ======================= host side =======================

def prep_inputs(inputs):
    bf = ml_dtypes.bfloat16
    tokens = np.asarray(inputs["tokens"])
    emb = np.asarray(inputs["emb"], np.float32)
    anw = np.asarray(inputs["attn_norm_w"], np.float32)
    Wq = np.asarray(inputs["Wq"], np.float32)
    Wk = np.asarray(inputs["Wk"], np.float32)
    Wv = np.asarray(inputs["Wv"], np.float32)
    Wo = np.asarray(inputs["Wo"], np.float32)
    ffw = np.asarray(inputs["ff_norm_w"], np.float32)
    W1 = np.asarray(inputs["W1"], np.float32)
    W2 = np.asarray(inputs["W2"], np.float32)
    W3 = np.asarray(inputs["W3"], np.float32)
    nfw = np.asarray(inputs["norm_f_w"], np.float32)

    Wq_s = Wq * anw[:, None, :]
    Wk_s = Wk * anw[:, None, :] / 8.0
    Wv_s = Wv * anw[:, None, :]
    W1_s = W1 * ffw[:, None, :]
    W2_s = W2 * ffw[:, None, :]
    emb_s = emb * nfw[None, :]

    pos = np.arange(T, dtype=np.float64)
    inv = 1.0 / (10000.0 ** (np.arange(32, dtype=np.float64) / 32.0))
    ang = pos[:, None] * inv[None, :]
    cos_fm = np.cos(ang).T.astype(np.float32)    # [32, T]
    sin_fm = np.sin(ang).T.astype(np.float32)
    rope = np.concatenate([cos_fm, sin_fm, cos_fm, sin_fm], 0).astype(bf)

    p_ = np.arange(128)[:, None]
    g_ = np.arange(896)[None, :] - 384
    dmask = np.where(p_ > g_, np.float32(0.0), np.float32(1.0)).astype(bf)

    toks = tokens.reshape(-1)
    in_maps = []
    for c in range(NCORES):
        wq_in = rearrange(Wq_s[:, 128 * c:128 * (c + 1), :],
                          "l m (k p) -> l p k m", p=128).astype(bf)
        kp = rearrange(Wk_s[:, 64 * c:64 * (c + 1), :],
                       "l m (k p) -> l p k m", p=128)
        vp = rearrange(Wv_s[:, 64 * c:64 * (c + 1), :],
                       "l m (k p) -> l p k m", p=128)
        wkv_in = np.concatenate([kp, vp], -1).astype(bf)
        wo_in = rearrange(Wo, "l (m mm) (cp p) -> l p cp m mm",
                          mm=128, p=128).astype(bf)
        w1p = rearrange(W1_s[:, 512 * c:512 * (c + 1), :],
                        "l (j jj) (k p) -> l p k j jj", jj=128, p=128)
        w2p = rearrange(W2_s[:, 512 * c:512 * (c + 1), :],
                        "l (j jj) (k p) -> l p k j jj", jj=128, p=128)
        w12_in = np.concatenate([w1p, w2p], 3).astype(bf)
        w3_in = rearrange(W3[:, :, 512 * c:512 * (c + 1)],
                          "l (m mm) (j p) -> l p j m mm", mm=128, p=128).astype(bf)
        embt_in = rearrange(emb_s[VLOC * c:VLOC * (c + 1), :],
                            "vv (k p) -> p k vv", p=128).astype(bf)
        x0 = emb[toks[TLOC * c:TLOC * (c + 1)]]
        x0_in = rearrange(x0, "(i p) cc -> i p cc", p=128).astype(np.float32)
        info = np.zeros((1, 32), np.int32)
        info[0, 0] = c * AGB
        info[0, 1] = c * TLOC
        info[0, 2] = c * AGB + 1024
        for d in range(8):
            dlog = PHYS_INV[PHYS[c] ^ d]
            info[0, 8 + d] = dlog * TLOC
            info[0, 16 + d] = c * AGB + dlog * TLOC
        in_maps.append({
            "wq": wq_in, "wkv": wkv_in, "wo": wo_in, "w12": w12_in,
            "w3": w3_in, "embt": embt_in, "x0": x0_in, "rope": rope,
            "dmask": dmask, "coreinfo": info,
        })
    return in_maps


def assemble(results):
    return np.concatenate([r["logits"] for r in results], axis=1)[None]


# ======================= harness entry point =======================

_CACHE = {}


def kernel(**inputs):
    """Full-model entry: takes unsharded inputs, returns [1, T, V] logits."""
    from concourse.bass_utils import run_bass_kernel_spmd
    if "nc" not in _CACHE:
        _CACHE["nc"] = build_nc()
    nc = _CACHE["nc"]
    in_maps = prep_inputs(inputs)
    res = run_bass_kernel_spmd(nc, in_maps, core_ids=list(range(NCORES)))
    return assemble(res.results).astype(np.float32)



# revision 19
# speedup vs baseline: 1.0451x; 1.0032x over previous
"""DeepSeek-style LM on 8 TRN2 cores — tensor-parallel Bass/Tile kernel.

Sharding: 2 Q heads + 1 KV head per core, HFF/8 per core, V/8 per core,
token-block/8 per core for residual+norm. Cross-core comm via
remote_dma_broadcast (SBUF->SBUF). Activations feature-major [C_part, T].

v2: pipelined communication — sends fire as soon as chunks are computed
(y2 per tau-pair, W3 reduce-scatter per slot in slot order), arrival
waits deferred to just before the first consumer. No credit handshake
(global round order transitively protects buffer reuse).
"""

import numpy as np
import ml_dtypes
from contextlib import ExitStack
from einops import rearrange

import concourse.bass as bass
import concourse.tile as tile
from concourse import bacc, mybir
from concourse.bass import ds
from concourse.masks import make_identity

F32 = mybir.dt.float32
BF16 = mybir.dt.bfloat16
I32 = mybir.dt.int32

NCORES = 8
T, C, DH, L, V, HFF = 2048, 1024, 64, 4, 32000, 4096
TLOC = T // NCORES          # 256
VLOC = V // NCORES          # 4000
EPS = 1e-5
AGB = 8 * TLOC              # 2048 elems: one sender's block in xh/rs_recv

# jax device index -> physical NeuronCore (tpb) index on this chip.
# remote_dma relative addressing XORs PHYSICAL tpb ids, so per-slot logical
# destinations are P^-1(P(c) ^ slot).
PHYS = [4, 5, 6, 7, 2, 3, 0, 1]
PHYS_INV = [PHYS.index(i) for i in range(8)]


def build_nc(taps=()):
    nc = bacc.Bacc("TRN2", target_bir_lowering=False, debug=False,
                   num_devices=NCORES)

    # ---------------- DRAM I/O ----------------
    dt = nc.dram_tensor
    wq_d = dt("wq", [L, 128, 8, 128], BF16, kind="ExternalInput").ap()
    wkv_d = dt("wkv", [L, 128, 8, 128], BF16, kind="ExternalInput").ap()
    wo_d = dt("wo", [L, 128, 8, 8, 128], BF16, kind="ExternalInput").ap()
    w12_d = dt("w12", [L, 128, 8, 8, 128], BF16, kind="ExternalInput").ap()
    w3_d = dt("w3", [L, 128, 4, 8, 128], BF16, kind="ExternalInput").ap()
    embt_d = dt("embt", [128, 8, VLOC], BF16, kind="ExternalInput").ap()
    x0_d = dt("x0", [2, 128, C], F32, kind="ExternalInput").ap()
    rope_d = dt("rope", [128, T], BF16, kind="ExternalInput").ap()
    dmask_d = dt("dmask", [128, 896], BF16, kind="ExternalInput").ap()
    info_d = dt("coreinfo", [1, 32], I32, kind="ExternalInput").ap()
    logits_d = dt("logits", [T, VLOC], F32, kind="ExternalOutput").ap()
    tap_d = {}
    for t_ in taps:
        shp = {"xh": [128, NCORES * AGB], "qfm": [128, T], "k2": [128, T],
               "vaug": [128, 16 * 65], "y2": [128, T],
               "xres": [128, 8 * TLOC], "hfm": [128, 4 * T],
               "rsrecv": [128, NCORES * AGB],
               "xres2": [128, 8 * TLOC]}[t_]
        dtp = F32 if t_ in ('xres', 'xres2') else BF16
        tap_d[t_] = dt("tap_" + t_, shp, dtp, kind="ExternalOutput").ap()

    # ---------------- static SBUF (fixed addrs for remote writes) -------
    xh = nc.alloc_sbuf_tensor("xh", [128, NCORES * AGB], BF16).ap()
    rs_recv = nc.alloc_sbuf_tensor("rs_recv", [128, NCORES * AGB], BF16).ap()
    info_sb = nc.alloc_sbuf_tensor("info_sb", [1, 32], I32).ap()

    xh4 = xh.rearrange("p (c k t) -> p c k t", c=NCORES, k=8)
    rsr = rs_recv.rearrange("p (c t) -> p c t", c=NCORES)

    # ---------------- semaphores ----------------
    sem = nc.alloc_semaphore
    lsem = sem("lsem")
    psem = sem("psem")
    dmas = sem("dmas")
    w3s = sem("w3sem")   # local-completion sem used ONLY by pair-0 W3 sends
    rsem = {k: sem(f"rsem_{k}") for k in ("xh", "rsb")}
    RD_ALL = [(0, k) for k in range(NCORES)]

    # cumulative counters (python-side bookkeeping of semaphore targets)
    state = dict(preps=0, lsem=0, w3s=0, rs={"xh": 0, "rsb": 0})

    def comm_send(kind, sends, name=""):
        """Issue remote broadcasts; no arrival wait. sends: [(src, dst, slot)].

        no_gpsimd_drain: the post-crit gpsimd drain would block until the
        remote DMA wire transfer completes (12-31us each). Buffer-reuse
        safety comes from explicit lsem waits at wA/wY/wR/wF instead."""
        gp = nc.gpsimd
        with tc.tile_critical(name=name, no_gpsimd_drain=True):
            for src, dst, slot in sends:
                rd = RD_ALL if slot is None else \
                    [(0, k) if k == slot else None for k in range(NCORES)]
                gp.remote_dma_broadcast(out_ap=dst, in_ap=src,
                                        remote_sem=rsem[kind], local_sem=lsem,
                                        rdests=rd).then_inc(psem, 1)
                state["preps"] += 1
                state["lsem"] += 16
                state["rs"][kind] += 2 if slot is not None else 16
            gp.wait_ge(psem, state["preps"])
            gp.trigger_dma(count=len(sends))

    def comm_wait(kind, target, lsem_thr=None, name=""):
        """Wait for arrivals (and optionally local send completion).
        Returns the crit's post-crit instruction for add_dep_helper edges
        (the scheduler treats crit bodies as opaque, so edges must target
        the post-crit barrier, not the inner wait)."""
        gp = nc.gpsimd
        with tc.tile_critical(name=name, no_gpsimd_drain=True):
            gp.wait_ge(rsem[kind], target)
            if lsem_thr is not None:
                gp.wait_ge(lsem, lsem_thr)
        return tc.prev_crit_insts[mybir.EngineType.Pool]

    def dep(consumer, waiter, why="comm arrival"):
        tile.add_dep_helper(consumer.ins, waiter, True, why)

    with tile.TileContext(nc) as tc, ExitStack() as ctx:
        # ---------- pools ----------
        sing = ctx.enter_context(tc.tile_pool(name="sing", bufs=1))
        spool = ctx.enter_context(tc.tile_pool(name="spool", bufs=1))
        spool1 = ctx.enter_context(tc.tile_pool(name="spool1", bufs=1))
        layer_ctx = ExitStack()
        act = layer_ctx.enter_context(tc.tile_pool(name="act", bufs=1))
        wpool = layer_ctx.enter_context(tc.tile_pool(name="wpool", bufs=2))
        wopool = layer_ctx.enter_context(tc.tile_pool(name="wopool", bufs=2))
        w8pool = layer_ctx.enter_context(tc.tile_pool(name="w8pool", bufs=1))
        w4pool = layer_ctx.enter_context(tc.tile_pool(name="w4pool", bufs=1))
        ppool = layer_ctx.enter_context(tc.tile_pool(name="ppool", bufs=2))
        prtpool = layer_ctx.enter_context(tc.tile_pool(name="prtpool", bufs=1))

        # ---------- constants ----------
        ident = sing.tile([128, 128], F32)
        make_identity(nc, ident)
        ident_bf = sing.tile([64, 64], BF16)
        nc.vector.tensor_copy(ident_bf[:], ident[0:64, 0:64])
        ones_sb = sing.tile([128, 128], BF16)
        nc.vector.memset(ones_sb, 1.0)
        rope_sb = sing.tile([128, T], BF16)
        nc.sync.dma_start(rope_sb[:], rope_d)
        eps_sb = sing.tile([128, 1], F32)
        nc.vector.memset(eps_sb, EPS)
        dmask_sb = sing.tile([128, 896], BF16)
        nc.sync.dma_start(dmask_sb[:], dmask_d)

        # persistent activations
        x_resid = sing.tile([128, 8, TLOC], F32)
        xh_send = sing.tile([128, 8, TLOC], BF16)
        xh_send_f = xh_send[:].rearrange("p k t -> p (k t)")
        x_resid_f = x_resid[:].rearrange("p k t -> p (k t)")

        # ---------- registers from coreinfo ----------
        regs = {}
        with tc.tile_critical():
            nc.gpsimd.dma_start(info_sb, info_d).then_inc(dmas, 16)
            nc.gpsimd.wait_ge(dmas, 16)

            def ld(eng, idx, mx):
                r = eng.alloc_register(f"r{idx}")
                eng.reg_load(r, info_sb[0:1, idx:idx + 1])
                return eng.snap(r, donate=True, min_val=0, max_val=mx)
            regs["xh_slot"] = ld(nc.gpsimd, 0, (NCORES - 1) * AGB)
            regs["y_slot"] = ld(nc.gpsimd, 1, (NCORES - 1) * TLOC)
            regs["xh_slot2"] = ld(nc.gpsimd, 2, (NCORES - 1) * AGB + 1024)
            regs["tok"] = [ld(nc.gpsimd, 8 + d, (NCORES - 1) * TLOC)
                           for d in range(8)]
            # y2 per-dest landing offsets: own_block_base + dest's y_slot
            regs["y2dst"] = [ld(nc.gpsimd, 16 + d,
                                (NCORES - 1) * AGB + (NCORES - 1) * TLOC)
                             for d in range(8)]
            # PE-engine copies for reg-offset moving APs (regs are per-engine)
            nc.tensor.wait_ge(dmas, 16)
            regs["y_slot_pe"] = ld(nc.tensor, 1, (NCORES - 1) * TLOC)
            regs["tok_pe"] = [ld(nc.tensor, 8 + d, (NCORES - 1) * TLOC)
                              for d in range(8)]

        # ---------- helpers ----------
        def norm_to_xh_send():
            """xh_send = rmsnorm(x_resid); first writer is a vector mul that
            is ordered behind the preceding comm_wait via vector-queue FIFO."""
            with tc.tile_pool(name="psn", bufs=1, space="PSUM") as psn:
                ns = psn.tile([128, TLOC], F32)
                for k in range(8):
                    sq = spool1.tile([128, TLOC], BF16, tag="sq")
                    nc.vector.tensor_mul(sq[:], x_resid[:, k, :], x_resid[:, k, :])
                    nc.tensor.matmul(ns[:], ones_sb[:], sq[:],
                                     start=(k == 0), stop=(k == 7))
                rms = spool1.tile([128, TLOC], F32, tag="rms")
                nc.scalar.activation(rms[:], ns[:],
                                     mybir.ActivationFunctionType.Sqrt,
                                     bias=eps_sb[:], scale=1.0 / C)
                rin = spool1.tile([128, TLOC], F32, tag="rin")
                nc.vector.reciprocal(rin[:], rms[:])
                for k in range(8):
                    nc.vector.tensor_mul(xh_send[:, k, :], x_resid[:, k, :], rin[:])
                    if k == 3:
                        comm_send("xh", [(xh_send_f[:, 0:1024],
                                          xh[:, ds(regs["xh_slot"], 1024)],
                                          None)], name="agA")

        def ag_send(name):
            comm_send("xh", [(xh_send_f[:, 1024:2048],
                              xh[:, ds(regs["xh_slot2"], 1024)], None)],
                      name=name)

        # ---------- x0 init ----------
        with tc.tile_pool(name="x0p", bufs=2) as x0p:
            with tc.tile_pool(name="ps0", bufs=2, space="PSUM") as ps0:
                for i in range(2):
                    x0_sb = x0p.tile([128, C], F32, tag="x0", name="x0_sb")
                    nc.sync.dma_start(x0_sb[:], x0_d[i])
                    for k in range(8):
                        tp = ps0.tile([128, 128], F32)
                        nc.tensor.transpose(tp[:], x0_sb[:, 128 * k:128 * (k + 1)],
                                            ident[:])
                        nc.vector.tensor_copy(x_resid[:, k, 128 * i:128 * (i + 1)],
                                              tp[:])
        norm_to_xh_send()
        ag_send("ag0")   # round 0: xh <- layer-0 attn input
        state.setdefault("ag_thr", []).append(state["rs"]["xh"])
        # lsem count covering the attn-AG sends: wY waits this before the
        # post-Wo norm rewrites xh_send (post-crit drains no longer cover it)
        state["lsem_attn"] = state["lsem"]

        def rope_apply(out_fm, ps, base, tau):
            """rotate-half on psum rows [base:base+64] -> out_fm bf16."""
            sl = slice(512 * tau, 512 * (tau + 1))
            cos = rope_sb[base:base + 32, sl]
            sin = rope_sb[base + 32:base + 64, sl]
            x1 = ps[base:base + 32, :]
            x2 = ps[base + 32:base + 64, :]
            t1 = spool.tile([32, 512], F32, tag="rt1")
            t2 = spool.tile([32, 512], F32, tag="rt2")
            nc.vector.tensor_mul(t1[:], x1, cos)
            nc.vector.tensor_mul(t2[:], x2, sin)
            nc.vector.tensor_sub(out_fm[base:base + 32, sl], t1[:], t2[:])
            nc.vector.tensor_mul(t1[:], x1, sin)
            nc.vector.tensor_mul(t2[:], x2, cos)
            nc.vector.tensor_add(out_fm[base + 32:base + 64, sl], t1[:], t2[:])

        # wq/wkv for layer 0 prefetched before the first wait
        wq_t = wpool.tile([128, 8, 128], BF16, tag="wq")
        nc.sync.dma_start(wq_t[:], wq_d[0])
        wkv_t = wpool.tile([128, 8, 128], BF16, tag="wkv")
        nc.sync.dma_start(wkv_t[:], wkv_d[0])

        # ================= layers =================
        for l in range(L):
            # ---- wait for this layer's attn all-gather ----
            wa = comm_wait("xh", state["ag_thr"][-1],
                           lsem_thr=state.get("lsem_prev"), name=f"wA{l}")
            q_fm = act.tile([128, T], BF16, tag="q_fm", name=f"q_fm{l}")
            k2_fm = act.tile([128, T], BF16, tag="k2_fm", name=f"k2_fm{l}")
            v_aug = act.tile([128, 16, 65], BF16, tag="v_aug", name=f"v_aug{l}")
            y2_send = act.tile([128, T], BF16, tag="y2_send", name=f"y2_send{l}")

            # prefetch Wo during QKV/attention (sync queue, before any crit
            # that could trap it behind the y wait; bufs=2 paces the stream)
            wo_t = []
            for cp in range(8):
                w_ = wopool.tile([128, 8, 128], BF16, tag="wo",
                                 name=f"wo_t{l}_{cp}")
                nc.sync.dma_start(w_[:], wo_d[l, :, cp])
                wo_t.append(w_)

            # ---- QKV matmuls with per-tau interleaving: v transposes run one
            # tau behind (v_stf copy has a full tau of slack), and per tau the
            # DVE stream is ordered v-path -> k-rope -> q-rope so attention's
            # earliest consumers (v_aug chunks, k2) resolve first. Late taus'
            # ropes drain into the attention phase where their consumers are.
            first_qkv = None
            v_stf = act.tile([64, T], BF16, tag="v_stf", name=f"v_stf{l}")
            nc.vector.memset(v_aug[:, :, 64:65], 1.0)
            with tc.tile_pool(name=f"psq{l}", bufs=2, space="PSUM") as P, \
                 tc.tile_pool(name=f"psk{l}", bufs=4, space="PSUM") as PK, \
                 tc.tile_pool(name=f"psv{l}", bufs=2, space="PSUM") as PV:
                def vtr_chunk(t2):
                    for i in range(4 * t2, 4 * t2 + 4):
                        vt = PV.tile([128, 64], BF16, tag="vtr")
                        nc.tensor.transpose(vt[:], v_stf[:, 128 * i:128 * (i + 1)],
                                            ident_bf[0:64, 0:64])
                        nc.vector.tensor_copy(v_aug[:, i, 0:64], vt[:])
                for tau in range(4):
                    rhs = xh4[:, 2 * tau:2 * tau + 2, :, :]
                    q_ps = P.tile([128, 512], F32, tag="q")
                    for k in range(8):
                        mm = nc.tensor.matmul(
                            q_ps[:], wq_t[:, k, :], rhs[:, :, k, :],
                            start=(k == 0), stop=(k == 7))
                        if first_qkv is None:
                            first_qkv = mm
                            dep(mm, wa, "attn AG arrival")
                    kv_ps = PK.tile([128, 512], F32, tag="k")
                    for k in range(8):
                        nc.tensor.matmul(
                            kv_ps[:], wkv_t[:, k, :], rhs[:, :, k, :],
                            start=(k == 0), stop=(k == 7))
                    if tau >= 1:
                        vtr_chunk(tau - 1)
                    nc.vector.tensor_copy(v_stf[:, 512 * tau:512 * (tau + 1)],
                                          kv_ps[64:128, :])
                    rope_apply(k2_fm, kv_ps, 0, tau)
                    # duplicate k rows for head 1 (scalar engine is idle here)
                    nc.scalar.copy(k2_fm[64:128, 512 * tau:512 * (tau + 1)],
                                   k2_fm[0:64, 512 * tau:512 * (tau + 1)])
                    rope_apply(q_fm, q_ps, 0, tau)
                    rope_apply(q_fm, q_ps, 64, tau)
                vtr_chunk(3)
            if l == 0 and "qfm" in tap_d:
                nc.sync.dma_start(tap_d["qfm"], q_fm[:])
            if l == 0 and "k2" in tap_d:
                nc.sync.dma_start(tap_d["k2"], k2_fm[:])
            if l == 0 and "vaug" in tap_d:
                nc.sync.dma_start(tap_d["vaug"],
                                  v_aug[:].rearrange("p a b -> p (a b)"))

            # prefetch W1/W2 during attention
            w12_t = [w8pool.tile([128, 8, 128], BF16, tag=f"w12_{k}",
                                 name=f"w12_t{l}_{k}") for k in range(8)]
            for k in range(8):
                nc.sync.dma_start(w12_t[k][:], w12_d[l, :, k])

            # ---- scores + softmax + AV; y2 broadcast per tau-pair ----
            with tc.tile_pool(name=f"psa{l}", bufs=2, space="PSUM") as B:
                for tau in range(4):
                    y_ps = [B.tile([65, 512], F32, tag=f"y{h}",
                                   name=f"y_ps{h}") for h in (0, 1)]
                    na = 4 * tau + 4
                    for a in range(na):
                        pts = []
                        for h in (0, 1):
                            s_ps = B.tile([128, 512], F32, tag=f"s{h}")
                            nc.tensor.matmul(
                                s_ps[:],
                                k2_fm[64 * h:64 * h + 64, 128 * a:128 * (a + 1)],
                                q_fm[64 * h:64 * h + 64, 512 * tau:512 * (tau + 1)],
                                start=True, stop=True)
                            p_t = ppool.tile([128, 512], BF16, tag=f"pT{h}")
                            nc.scalar.activation(p_t[:], s_ps[:],
                                                 mybir.ActivationFunctionType.Exp)
                            if a >= 4 * tau:
                                r_ = a - 4 * tau
                                nc.vector.tensor_mul(
                                    p_t[:], p_t[:],
                                    dmask_sb[:, 384 - 128 * r_:896 - 128 * r_])
                            pts.append(p_t)
                        for h in (0, 1):
                            nc.tensor.matmul(y_ps[h][:], v_aug[:, a, :], pts[h][:],
                                             start=(a == 0), stop=(a == na - 1))
                    # softmax denominators: batch both heads in one reciprocal
                    # (partition_broadcast needs base-0 in/out; gpsimd copies
                    # shift rows to base 0 first)
                    den = spool1.tile([33, 512], F32, tag="den")
                    for h in (0, 1):
                        nc.vector.tensor_copy(den[32 * h:32 * h + 1, :],
                                              y_ps[h][64:65, :])
                    nc.vector.reciprocal(den[:], den[:])
                    r1 = spool.tile([32, 512], F32, tag="rt1")
                    nc.vector.tensor_copy(r1[0:1, :], den[32:33, :])
                    rdb = [spool1.tile([64, 512], F32, tag=f"rdb{h}",
                                       name=f"rdb{h}") for h in (0, 1)]
                    nc.gpsimd.partition_broadcast(rdb[0][:], den[0:1, :])
                    nc.gpsimd.partition_broadcast(rdb[1][:], r1[0:1, :])
                    for h in (0, 1):
                        nc.vector.tensor_mul(
                            y2_send[64 * h:64 * h + 64, 512 * tau:512 * (tau + 1)],
                            y_ps[h][0:64, :], rdb[h][:])

            if l == 0 and "y2" in tap_d:
                nc.sync.dma_start(tap_d["y2"], y2_send[:])

            # ---- per-dest y2 reduce-scatter: dest d only needs its own
            # 256-token slice (wire 0.5MB vs 3.5MB broadcast). Slice and dst
            # offsets are register-indexed (slot->token-block map is per-core).
            gp = nc.gpsimd
            with tc.tile_critical(name=f"y2s{l}", no_gpsimd_drain=True):
                for d in range(8):
                    rd = [(0, k) if k == d else None for k in range(NCORES)]
                    gp.remote_dma_broadcast(
                        out_ap=rs_recv[:, ds(regs["y2dst"][d], TLOC)],
                        in_ap=y2_send[:, ds(regs["tok"][d], TLOC)],
                        remote_sem=rsem["rsb"], local_sem=lsem,
                        rdests=rd).then_inc(psem, 1)
                    state["preps"] += 1
                    state["lsem"] += 16
                    state["rs"]["rsb"] += 2
                gp.wait_ge(psem, state["preps"])
                gp.trigger_dma(count=8)

            # ---- wait all y2, then Wo directly from rs_recv (reg offset) ----
            # lsem_thr must be the FULL issued count: lsem increments arrive
            # out of order across engines, so a partial-count threshold can be
            # satisfied by later sends' lanes while earlier lanes still read.
            wy = comm_wait("rsb", state["rs"]["rsb"],
                           lsem_thr=state["lsem"], name=f"wY{l}")
            first_wo = None
            with tc.tile_pool(name=f"psw{l}", bufs=1, space="PSUM") as W:
                wo_ps = [W.tile([128, TLOC], F32, tag=f"wo{m}",
                                name=f"wo_ps{m}") for m in range(8)]
                for cp in range(8):
                    for m in range(8):
                        mm = nc.tensor.matmul(
                            wo_ps[m][:], wo_t[cp][:, m, :],
                            rsr[:, cp, ds(regs["y_slot_pe"], TLOC)],
                            start=(cp == 0), stop=(cp == 7))
                        if first_wo is None:
                            first_wo = mm
                            dep(mm, wy, "y2 arrival")
                for m in range(8):
                    nc.vector.tensor_add(x_resid[:, m, :], x_resid[:, m, :],
                                         wo_ps[m][:])
            if l == 0 and "xres" in tap_d:
                nc.sync.dma_start(tap_d["xres"], x_resid_f)

            # ---- norm + AG for MLP ----
            norm_to_xh_send()
            ag_send(f"agM{l}")
            state["ag_thr"].append(state["rs"]["xh"])

            # prefetch W3 during MLP
            w3_t = [w4pool.tile([128, 8, 128], BF16, tag=f"w3_{j}",
                                name=f"w3_t{l}_{j}") for j in range(4)]
            for j in range(4):
                nc.sync.dma_start(w3_t[j][:], w3_d[l, :, j])

            wm = comm_wait("xh", state["ag_thr"][-1], name=f"wM{l}")
            h_fm = act.tile([128, 4, T], BF16, tag="h_fm", name=f"h_fm{l}")
            # ---- MLP W1/W2 ----
            first_mlp = None
            with tc.tile_pool(name=f"psm{l}", bufs=2, space="PSUM") as M:
                for j in range(4):
                    for tau in range(4):
                        rhs = xh4[:, 2 * tau:2 * tau + 2, :, :]
                        a_ps = M.tile([128, 512], F32, tag="aps")
                        b_ps = M.tile([128, 512], F32, tag="bps")
                        for k in range(8):
                            mm = nc.tensor.matmul(
                                a_ps[:], w12_t[k][:, j, :], rhs[:, :, k, :],
                                start=(k == 0), stop=(k == 7))
                            if first_mlp is None:
                                first_mlp = mm
                                dep(mm, wm, "MLP AG arrival")
                        for k in range(8):
                            nc.tensor.matmul(
                                b_ps[:], w12_t[k][:, 4 + j, :], rhs[:, :, k, :],
                                start=(k == 0), stop=(k == 7))
                        sil = spool1.tile([128, 512], BF16, tag="sil")
                        nc.scalar.activation(sil[:], a_ps[:],
                                             mybir.ActivationFunctionType.Silu)
                        nc.vector.tensor_mul(h_fm[:, j, 512 * tau:512 * (tau + 1)],
                                             sil[:], b_ps[:])
            if l == 0 and "hfm" in tap_d:
                nc.sync.dma_start(tap_d["hfm"], h_fm[:].rearrange("p a b -> p (a b)"))

            # ---- W3 partials per slot (slot order), send as computed ----
            # 6 prt buffers; d=6,7 reuse pair-0's buffers. Pair-0 sends use
            # the dedicated w3s local sem, so the reuse wait is a FULL-count
            # threshold on w3s (race-free); all other sends use lsem and are
            # covered by wR's full-count lsem threshold.
            w6_wait = None
            with tc.tile_pool(name=f"ps3{l}", bufs=2, space="PSUM") as W3P:
                pend = []
                for d in range(8):
                    p3 = W3P.tile([128, 8, TLOC], F32, tag="w3p")
                    # m pairs (2m, 2m+1) share a 2KB PSUM bank and
                    # start_tensor_calc zeroes the WHOLE bank: only the
                    # even-m j=0 matmul may carry start=True.
                    for j in range(4):
                        for m in range(8):
                            nc.tensor.matmul(
                                p3[:, m, :], w3_t[j][:, m, :],
                                h_fm[:, j, ds(regs["tok_pe"][d], TLOC)],
                                start=(j == 0 and m % 2 == 0), stop=(j == 3),
                                skip_group_check=(m % 2 == 1))
                    prt = prtpool.tile([128, 8, TLOC], BF16, tag=f"prt{d % 6}")
                    ci = nc.vector.tensor_copy(prt[:], p3[:])
                    if d >= 6:
                        dep(ci, w6_wait, "prt buffer reused after send drained")
                    pend.append((prt[:].rearrange("p m t -> p (m t)"),
                                 rs_recv[:, ds(regs["xh_slot"], AGB)], d))
                    if d % 2 == 1:
                        pair0 = (d == 1)
                        lls = w3s if pair0 else lsem
                        gp = nc.gpsimd
                        with tc.tile_critical(name=f"rs{l}_{d // 2}",
                                              no_gpsimd_drain=True):
                            for src, dst, slot in pend:
                                rd = [(0, k) if k == slot else None
                                      for k in range(NCORES)]
                                gp.remote_dma_broadcast(
                                    out_ap=dst, in_ap=src,
                                    remote_sem=rsem["rsb"], local_sem=lls,
                                    rdests=rd).then_inc(psem, 1)
                                state["preps"] += 1
                                state["w3s" if pair0 else "lsem"] += 16
                                state["rs"]["rsb"] += 2
                            gp.wait_ge(psem, state["preps"])
                            gp.trigger_dma(count=len(pend))
                            if d == 5:
                                gp.wait_ge(w3s, state["w3s"])
                        if d == 5:
                            w6_wait = tc.prev_crit_insts[mybir.EngineType.Pool]
                        pend = []

            # ---- wait RS arrivals; residual add tree on vector ----
            wr = comm_wait("rsb", state["rs"]["rsb"],
                           lsem_thr=state["lsem"], name=f"wR{l}")
            t_a = spool1.tile([128, AGB], BF16, tag="racc0")
            a0 = nc.vector.tensor_add(
                t_a[:], rs_recv[:, 0:AGB], rs_recv[:, AGB:2 * AGB])
            dep(a0, wr, "RS arrival")
            for s_ in range(2, 7):
                nc.vector.tensor_add(t_a[:], t_a[:],
                                     rs_recv[:, AGB * s_:AGB * (s_ + 1)])
            nc.vector.tensor_add(x_resid_f, x_resid_f, t_a[:])
            nc.vector.tensor_add(x_resid_f, x_resid_f,
                                 rs_recv[:, 7 * AGB:8 * AGB])
            if l == 0 and "rsrecv" in tap_d:
                nc.sync.dma_start(tap_d["rsrecv"], rs_recv)
            if l == 0 and "xres2" in tap_d:
                nc.sync.dma_start(tap_d["xres2"], x_resid_f)

            # prefetch next layer's wq/wkv before the next wait
            if l + 1 < L:
                wq_t = wpool.tile([128, 8, 128], BF16, tag="wq")
                nc.sync.dma_start(wq_t[:], wq_d[l + 1])
                wkv_t = wpool.tile([128, 8, 128], BF16, tag="wkv")
                nc.sync.dma_start(wkv_t[:], wkv_d[l + 1])

            # ---- norm + AG for next layer / final ----
            norm_to_xh_send()
            ag_send(f"agN{l}")
            state["lsem_prev"] = state["lsem"]
            state["lsem_attn"] = state["lsem"]
            state["ag_thr"].append(state["rs"]["xh"])

        # ================= lm head =================
        layer_ctx.close()
        with tc.tile_pool(name="embp", bufs=8) as embp, \
             tc.tile_pool(name="outp", bufs=4) as outp, \
             tc.tile_pool(name="pslm", bufs=2, space="PSUM") as LM:
            embt = [embp.tile([128, VLOC], BF16, tag="embt", name="embt_t")
                    for _ in range(8)]
            for k in range(8):
                nc.sync.dma_start(embt[k][:], embt_d[:, k, :])
            wf = comm_wait("xh", state["ag_thr"][-1],
                           lsem_thr=state["lsem"], name="wF")
            first_lm = None
            # 4 vocab tiles x bufs=2: set s's copies overlap set s+1's matmuls
            for i in range(16):
                cpr, half = i // 2, i % 2
                for s in range(2):
                    lm_ps = [LM.tile([128, 500], F32, tag=f"lm{v}",
                                     name=f"lm_ps{v}") for v in range(4)]
                    for k in range(8):
                        lh = xh[:, cpr * AGB + k * TLOC + half * 128:
                                cpr * AGB + k * TLOC + half * 128 + 128]
                        for v in range(4):
                            vv = 4 * s + v
                            mm = nc.tensor.matmul(
                                lm_ps[v][:], lh,
                                embt[k][:, 500 * vv:500 * (vv + 1)],
                                start=(k == 0), stop=(k == 7))
                            if first_lm is None:
                                first_lm = mm
                                dep(mm, wf, "final AG arrival")
                    for v in range(4):
                        vv = 4 * s + v
                        o = outp.tile([128, 500], F32, tag="o")
                        if v % 2 == 0:
                            nc.vector.tensor_copy(o[:], lm_ps[v][:])
                        else:
                            nc.scalar.copy(o[:], lm_ps[v][:])
                        nc.sync.dma_start(
                            logits_d[128 * i:128 * (i + 1),
                                     500 * vv:500 * (vv + 1)], o[:])

    nc.compile()
    return nc


# ======================= host side =======================

def prep_inputs(inputs):
    bf = ml_dtypes.bfloat16
    tokens = np.asarray(inputs["tokens"])
    emb = np.asarray(inputs["emb"], np.float32)
    anw = np.asarray(inputs["attn_norm_w"], np.float32)
    Wq = np.asarray(inputs["Wq"], np.float32)
    Wk = np.asarray(inputs["Wk"], np.float32)
    Wv = np.asarray(inputs["Wv"], np.float32)
    Wo = np.asarray(inputs["Wo"], np.float32)
    ffw = np.asarray(inputs["ff_norm_w"], np.float32)
    W1 = np.asarray(inputs["W1"], np.float32)
    W2 = np.asarray(inputs["W2"], np.float32)
    W3 = np.asarray(inputs["W3"], np.float32)
    nfw = np.asarray(inputs["norm_f_w"], np.float32)

    Wq_s = Wq * anw[:, None, :]
    Wk_s = Wk * anw[:, None, :] / 8.0
    Wv_s = Wv * anw[:, None, :]
    W1_s = W1 * ffw[:, None, :]
    W2_s = W2 * ffw[:, None, :]
    emb_s = emb * nfw[None, :]

    pos = np.arange(T, dtype=np.float64)
    inv = 1.0 / (10000.0 ** (np.arange(32, dtype=np.float64) / 32.0))
    ang = pos[:, None] * inv[None, :]
    cos_fm = np.cos(ang).T.astype(np.float32)    # [32, T]
    sin_fm = np.sin(ang).T.astype(np.float32)
    rope = np.concatenate([cos_fm, sin_fm, cos_fm, sin_fm], 0).astype(bf)

    p_ = np.arange(128)[:, None]
    g_ = np.arange(896)[None, :] - 384
    dmask = np.where(p_ > g_, np.float32(0.0), np.float32(1.0)).astype(bf)

    toks = tokens.reshape(-1)
    in_maps = []
    for c in range(NCORES):
        wq_in = rearrange(Wq_s[:, 128 * c:128 * (c + 1), :],
                          "l m (k p) -> l p k m", p=128).astype(bf)
        kp = rearrange(Wk_s[:, 64 * c:64 * (c + 1), :],
                       "l m (k p) -> l p k m", p=128)
        vp = rearrange(Wv_s[:, 64 * c:64 * (c + 1), :],
                       "l m (k p) -> l p k m", p=128)
        wkv_in = np.concatenate([kp, vp], -1).astype(bf)
        wo_in = rearrange(Wo, "l (m mm) (cp p) -> l p cp m mm",
                          mm=128, p=128).astype(bf)
        w1p = rearrange(W1_s[:, 512 * c:512 * (c + 1), :],
                        "l (j jj) (k p) -> l p k j jj", jj=128, p=128)
        w2p = rearrange(W2_s[:, 512 * c:512 * (c + 1), :],
                        "l (j jj) (k p) -> l p k j jj", jj=128, p=128)
        w12_in = np.concatenate([w1p, w2p], 3).astype(bf)
        w3_in = rearrange(W3[:, :, 512 * c:512 * (c + 1)],
                          "l (m mm) (j p) -> l p j m mm", mm=128, p=128).astype(bf)
        embt_in = rearrange(emb_s[VLOC * c:VLOC * (c + 1), :],
                            "vv (k p) -> p k vv", p=128).astype(bf)
        x0 = emb[toks[TLOC * c:TLOC * (c + 1)]]
        x0_in = rearrange(x0, "(i p) cc -> i p cc", p=128).astype(np.float32)
        info = np.zeros((1, 32), np.int32)
        info[0, 0] = c * AGB
        info[0, 1] = c * TLOC
        info[0, 2] = c * AGB + 1024
        for d in range(8):
            dlog = PHYS_INV[PHYS[c] ^ d]
            info[0, 8 + d] = dlog * TLOC
            info[0, 16 + d] = c * AGB + dlog * TLOC
        in_maps.append({
            "wq": wq_in, "wkv": wkv_in, "wo": wo_in, "w12": w12_in,
            "w3": w3_in, "embt": embt_in, "x0": x0_in, "rope": rope,
            "dmask": dmask, "coreinfo": info,
        })
    return in_maps


def assemble(results):
    return np.concatenate([r["logits"] for r in results], axis=1)[None]


# ======================= harness entry point =======================

_CACHE = {}


def kernel(**inputs):
    """Full-model entry: takes unsharded inputs, returns [1, T, V] logits."""
    from concourse.bass_utils import run_bass_kernel_spmd
    if "nc" not in _CACHE:
        _CACHE["nc"] = build_nc()
    nc = _CACHE["nc"]
    in_maps = prep_inputs(inputs)
    res = run_bass_kernel_spmd(nc, in_maps, core_ids=list(range(NCORES)))
    return assemble(res.results).astype(np.float32)

